# revision 33
# baseline (speedup 1.0000x reference)
"""Trainium2 Bass kernel for nn_DFPT_Node (soft binary decision tree).

Full inputs in, full output out; internally data-parallel over 8 NeuronCores
(batch sharded 65536 -> 8 x 8192). Tree params (c, s, dims, leaf_logits) are
baked into compiled constants on the host.

  gate:  z = a*x + b via a scaled one-hot matmul with K=128 =
         [x_hi(64) | x_lo(62 dims) | 1 | 1]; rows 126/127 carry b = b_hi+b_lo
         (f16 split, ~22 bits). Chunks 0-5 emit z and ACT applies sigmoid.
         Chunks 6-7 (level-9 right-children) emit z/T and are consumed in
         tanh form tau = 2*sigma-1: either ACT Tanh(z/2) (exact pieces) or a
         clamped odd quintic tau~ = t*(PA + PB*u + PC*u^2), u = t^2,
         t = clamp(z/T, +-1), built from stock DVE/Pool ops (sigma err 7e-3,
         end-to-end rel err ~5e-3, budget 2e-2).  This moves ~12K of the 64K
         sigmoid elems/partition off the ACT engine (the bottleneck) onto
         DVE tensor_scalar (4x mode) + Pool (flat 0.83 ns/elem) capacity.
  tree:  levels 0-6 batch-major pair-wide (two slabs per cascade), levels
         7-9 node-major pair-wide [128, 2048] tiles.  Fold basis per pair:
         F = [l8, r8, l9a, l9b, m_a, m_b, m_c, m_d] where m_a = l9a*g4,
         m_b = l9b*g5 (sigma form) and m_c = r9a*tau6, m_d = r9b*tau7
         (tau form); M rows absorb the (L_l+L_r)/2, (L_l-L_r)/2 halves.
  fold:  flipped matmuls: out[128 batch, 16] += F[:,w*128:(w+1)*128].T-style
         accumulation (lhsT = F window, rhs = M chunk [128,16]) -> 13ns/mm
         on PE instead of 213ns, PSUM = 2 banks for fold + 2x3 banks z.
  sched: event-driven pipeline at supertile granularity; per-pair piece
         order ch 1,2,6,3,7,4,5 so the poly chains run mid-pair and the
         tail is a short tensor op + 4 fold matmuls + evac.

Output leaves the device as outt [128, 1024] f16: partition p holds batch
row (2*pair+si)*1024 + j*128 + p at column pair*256 + si*128 + j*16 + cl.
"""

import numpy as np

B_FULL = 65536
IN_DIM = 64
N_CLASS = 10
MAX_DEPTH = 10
N_CORES = 8
B_CORE = B_FULL // N_CORES      # 8192
SLAB = 1024                     # batch columns per slab
N_SLABS = B_CORE // SLAB        # 8
N_CHUNKS = 8                    # node-major chunks of 128 nodes
N_PAIRS = N_SLABS // 2
SUPER = 1536

# clamped odd-quintic tanh(z/2) approximation on z/T
T_POLY = 4.2715
PA = 2.02125
PB = -1.89797
PC = 0.86295

# pieces of chunks 6/7 evaluated by the polynomial (rest: ACT Tanh).
# (ch, si, h) per pair; tune for ACT vs DVE/Pool balance.
POLY_PIECES = {(6, 0, 0), (6, 0, 1)}

# per-pair deep piece order (ch, si, h): poly pieces spread through the
# pair so ACT never runs dry; each chunk's (1,1) piece is ordered last.
PIECE_ORDER = (
    [(1, si, h) for si in range(2) for h in range(2)]
    + [(2, si, h) for si in range(2) for h in range(2)]
    + [(6, 0, 0), (6, 0, 1)]
    + [(3, si, h) for si in range(2) for h in range(2)]
    + [(6, 1, 0), (6, 1, 1)]
    + [(7, si, h) for si in range(2) for h in range(2)]
    + [(4, si, h) for si in range(2) for h in range(2)]
    + [(5, si, h) for si in range(2) for h in range(2)]
)
IDXE = {1: 0, 2: 1, 3: 2, 7: 3, 4: 4, 5: 5}   # exact chunk -> gtd slot

F16 = np.float16
F32 = np.float32

_CACHE = {}
_DEBUG_TAP = None     # ("tilename", pair, col0) -> dumps tile[:, col0:col0+1024]


def _build_tree_layout():
    pos = [np.array([0], dtype=np.int64)]
    for _ in range(MAX_DEPTH):
        p = pos[-1]
        pos.append(np.concatenate([2 * p, 2 * p + 1]))
    return pos


def _chunk_nodes():
    pos = _build_tree_layout()
    chunk_nodes = -np.ones((N_CHUNKS, 128), dtype=np.int64)
    for d in range(7):
        base = (1 << d) - 1
        chunk_nodes[0, base: base + (1 << d)] = base + pos[d]
    chunk_nodes[1, :] = 127 + pos[7]
    lvl8 = 255 + pos[8]
    chunk_nodes[2, :] = lvl8[:128]
    chunk_nodes[3, :] = lvl8[128:]
    lvl9 = 511 + pos[9]
    for t in range(4):
        chunk_nodes[4 + t, :] = lvl9[128 * t: 128 * (t + 1)]
    return chunk_nodes, pos


def _build_constants(c, s, dims, leaf_logits):
    """W chunks [8,128,128] f16 (bias in rows 126/127; chunks 6/7 pre-scaled
    by 1/T_POLY), M [8,128,16] f16 for the flipped fold."""
    chunk_nodes, pos = _chunk_nodes()

    a64 = -4.0 / np.abs(s.astype(np.float64))
    a16 = a64.astype(F16)

    # two dims whose sharpest gate is softest lose their x_lo row
    min_s = np.full(IN_DIM, np.inf)
    for g in range(len(dims)):
        d = int(dims[g])
        min_s[d] = min(min_s[d], abs(float(s[g])))
    drop = np.argsort(-min_s)[:2]
    lo_row = {}
    r = IN_DIM
    for d in range(IN_DIM):
        if d not in drop:
            lo_row[d] = r
            r += 1
    assert r == 126

    W = np.zeros((N_CHUNKS, 128, 128), dtype=F16)
    ch_idx, col_idx = np.nonzero(chunk_nodes >= 0)
    g_idx = chunk_nodes[ch_idx, col_idx]
    for ch, col, g in zip(ch_idx, col_idx, g_idx):
        scale = (1.0 / T_POLY) if ch == 6 else 1.0
        aS = np.float16(a16[g].astype(np.float64) * scale)
        bS64 = -aS.astype(np.float64) * float(c[g])
        b_hi = np.float16(bS64)
        b_lo = np.float16(bS64 - b_hi.astype(np.float64))
        d = int(dims[g])
        W[ch, d, col] = aS
        if d in lo_row:
            W[ch, lo_row[d], col] = aS
        W[ch, 126, col] = b_hi
        W[ch, 127, col] = b_lo

    L_my = leaf_logits[pos[MAX_DEPTH]].astype(np.float64)  # [1024, 10] block
    A = L_my[:512] - L_my[512:]          # L_left - L_right per level-9 node
    Bm = L_my[512:]                      # L_right
    At = [A[128 * t: 128 * (t + 1)] for t in range(4)]
    Bt = [Bm[128 * t: 128 * (t + 1)] for t in range(4)]
    S2 = Bt[2] + At[2] / 2               # (L_l+L_r)/2 for chunk-6 children
    # F = [l8, r8, l9a, l9b, m_a, m_b, m_c(tau), m_d(sigma)]
    Mlist = [S2, Bt[3], Bt[0] - S2, Bt[1] - Bt[3],
             At[0], At[1], At[2] / 2, At[3]]
    M = np.zeros((N_CHUNKS, 128, 16), dtype=F16)
    for i, m in enumerate(Mlist):
        M[i, :, :N_CLASS] = m.astype(F16)
    return W, M, lo_row


def _gt0_steps():
    """Phase-1: chunk 0 of every slab into gt0. (dest, off, width, pieces)"""
    steps = []
    p = 0
    off = 0
    for w in (512, 1536, 1536, 1536, 1536, 1536):
        n = w // 512
        pieces = [(q // 2, q % 2) for q in range(p, p + n)]   # (slab, half)
        steps.append((off, w, pieces))
        p += n
        off += w
    return steps


def _build_program():
    import concourse.bass as bass
    import concourse.tile as tile
    from concourse import bacc, mybir

    f16 = mybir.dt.float16
    f32 = mybir.dt.float32
    SIG = mybir.ActivationFunctionType.Sigmoid
    TANH = mybir.ActivationFunctionType.Tanh
    MAX = mybir.AluOpType.max
    MIN = mybir.AluOpType.min
    MULT = mybir.AluOpType.mult
    ADD = mybir.AluOpType.add

    nc = bacc.Bacc("TRN2", target_bir_lowering=False)
    xt2_d = nc.dram_tensor("xt2", [128, B_CORE], f16, kind="ExternalInput")
    wt_d = nc.dram_tensor("wt", [128, N_CHUNKS, 128], f16, kind="ExternalInput")
    mt_d = nc.dram_tensor("mt", [128, N_CHUNKS, 16], f16, kind="ExternalInput")
    out_d = nc.dram_tensor("outt", [128, N_SLABS * 128], f16,
                           kind="ExternalOutput")

    with tile.TileContext(nc) as tc:
        with (
            tc.tile_pool(name="singles", bufs=1) as singles,
            tc.tile_pool(name="gtpool", bufs=2) as gtpool,
            tc.tile_pool(name="tpool", bufs=3) as tpool,
            tc.tile_pool(name="chain", bufs=2) as chain,
            tc.tile_pool(name="work", bufs=2) as work,
            tc.tile_pool(name="fwork", bufs=1) as fwork,
            tc.tile_pool(name="zpsum", bufs=2, space="PSUM") as zpsum,
            tc.tile_pool(name="opsum", bufs=2, space="PSUM") as opsum,
        ):
            # resident constants; ordered so step 0's z can start ASAP
            w_sb = singles.tile([128, N_CHUNKS, 128], f16)
            nc.sync.dma_start(out=w_sb[:, 0:1, :], in_=wt_d[:, 0:1, :])
            xt2 = singles.tile([128, B_CORE], f16)
            nc.scalar.dma_start(out=xt2[:, 0:512], in_=xt2_d[:, 0:512])
            nc.sync.dma_start(out=xt2[:, 512:2048], in_=xt2_d[:, 512:2048])
            nc.sync.dma_start(out=w_sb[:, 1:, :], in_=wt_d[:, 1:, :])
            ones0 = singles.tile([128, 16, 1], f16)
            nc.vector.memset(ones0, 1.0)
            # PE p-state warmup from t~0
            pewarm = singles.tile([128, 512], f16)
            nc.vector.memset(pewarm, 0.0)
            for _ in range(4):
                zw = zpsum.tile([128, SUPER], f32, tag="zs", name="zwarm")
                nc.tensor.matmul(zw[:, 0:512], lhsT=pewarm[:, 0:128],
                                 rhs=pewarm, start=True, stop=True)
            sigwarm = singles.tile([1, 1], f16)
            nc.vector.memset(sigwarm, 0.0)
            nc.scalar.activation(sigwarm, sigwarm, SIG, bias=0.0, scale=1.0)
            m_sb = singles.tile([128, N_CHUNKS, 16], f16)
            nc.sync.dma_start(out=m_sb, in_=mt_d[:, :, :])
            gt0 = singles.tile([128, N_SLABS * SLAB], f16)

            state = {}

            def emit_shallow(s):
                # chunk-0 gates batch-major via DMA xbar transpose
                pair = s // 2
                if ("gsh", pair) not in state:
                    state[("gsh", pair)] = work.tile(
                        [128, 16, 128], f16, tag="gsh", name=f"gsh{pair}")
                gsh = state[("gsh", pair)]
                for g in range(8):
                    nc.sync.dma_start(
                        out=gsh[:, (s % 2) * 8 + g, :],
                        in_=gt0[:, s * SLAB + g * 128: s * SLAB + (g + 1) * 128],
                        transpose=True,
                    )
                state[("shdone", s)] = True
                if state.get(("shdone", s ^ 1)):
                    emit_cascade(pair)

            def emit_cascade(pair):
                # shallow cascade pair-wide (batch-major, 16 groups)
                gsh = state[("gsh", pair)]
                prev = ones0[:, :, :]
                for d in range(7):
                    n = 1 << d
                    cur = work.tile([128, 16, 2 * n], f16, tag=f"pb{d + 1}",
                                    name=f"pb{d + 1}_{pair}")
                    gl = gsh[:, :, n - 1: 2 * n - 1]
                    nc.vector.tensor_mul(cur[:, :, 0:n], prev, gl)
                    nc.vector.tensor_sub(cur[:, :, n: 2 * n], prev,
                                         cur[:, :, 0:n])
                    prev = cur
                # batch-major -> node-major via DMA xbar transpose
                p7t = work.tile([128, 2 * SLAB], f16, tag="p7t",
                                name=f"p7t{pair}")
                for g in range(16):
                    nc.sync.dma_start(
                        out=p7t[:, g * 128: (g + 1) * 128],
                        in_=prev[:, g, :],
                        transpose=True,
                    )
                state[("p7t", pair)] = p7t
                tap("p7t", pair, p7t)

            def gtE(pair, ch):
                gt = state[("gtd", pair)]
                base = IDXE[ch] * 2 * SLAB
                return gt[:, base: base + 2 * SLAB]

            def ft(name, pair, width=2 * SLAB):
                t = fwork.tile([128, width], f16, tag=name,
                               name=f"{name}_{pair}")
                state[(name, pair)] = t
                return t

            fold_q = []

            def flush_folds():
                for fn, args in fold_q:
                    fn(*args)
                fold_q.clear()

            def emit_fold(pair, f, src, regions=None):
                fold_q.append((emit_fold_now, (pair, f, src, regions)))

            def emit_fold_now(pair, f, src, regions=None):
                # flipped fold: for each (si, j) window accumulate
                # op[:, si, j, :] += src[:, si*1024+j*128 : +128].T @ M[f]
                if ("op", pair) not in state:
                    state[("op", pair)] = opsum.tile(
                        [128, 2, 8, 16], f32, tag="op", name=f"op{pair}")
                    # pre-zero; all fold matmuls pure-accumulate (psum start
                    # would zero the whole 2KB zero-region, wiping siblings)
                    nc.vector.memset(state[("op", pair)], 0.0)
                op = state[("op", pair)]
                if regions is None:
                    regions = [(si, j) for si in range(2) for j in range(8)]
                for si, j in regions:
                    nc.tensor.matmul(
                        op[:, si, j, :],
                        lhsT=src[:, si * SLAB + j * 128: si * SLAB + (j + 1) * 128],
                        rhs=m_sb[:, f, :],
                        start=False,
                        stop=False,
                        skip_group_check=True,
                    )

            def tap(name, pair, tile):
                if _DEBUG_TAP and _DEBUG_TAP[0] == name and _DEBUG_TAP[1] == pair:
                    c0 = _DEBUG_TAP[2]
                    nc.sync.dma_start(out=out_d[:, 0:1024],
                                      in_=tile[:, c0:c0 + 1024])

            def emit_out_half(pair, si, h):
                if _DEBUG_TAP:
                    return
                op = state[("op", pair)]
                osb = work.tile([128, 64], f16, tag="osbh",
                                name=f"osbh{pair}_{si}_{h}")
                nc.scalar.copy(osb, op[:, si, h * 4: h * 4 + 4, :])
                c0 = (2 * pair + si) * 128 + h * 64
                nc.scalar.dma_start(out=out_d[:, c0: c0 + 64], in_=osb)

            def emit_out(pair, si):
                if _DEBUG_TAP:
                    return
                op = state[("op", pair)]
                osb = work.tile([128, 128], f16, tag="osb",
                                name=f"osb{pair}_{si}")
                nc.vector.tensor_scalar(osb, op[:, si, :, :], 1.0, 0.0,
                                        MULT, ADD)
                q = nc.sync
                q.dma_start(
                    out=out_d[:, (2 * pair + si) * 128: (2 * pair + si + 1) * 128],
                    in_=osb,
                )

            def poly_runs(pair, ch):
                """maximal contiguous runs of poly pieces in t-tile offsets"""
                runs = []
                cur = None
                for si in range(2):
                    for h in range(2):
                        off = si * SLAB + h * 512
                        if (ch, si, h) in POLY_PIECES:
                            if cur is not None and cur[1] == off:
                                cur[1] = off + 512
                            else:
                                cur = [off, off + 512]
                                runs.append(cur)
                        else:
                            cur = None
                return [(a, b) for a, b in runs]

            def exact_runs(pair, ch):
                runs = []
                cur = None
                for si in range(2):
                    for h in range(2):
                        off = si * SLAB + h * 512
                        if (ch, si, h) not in POLY_PIECES:
                            if cur is not None and cur[1] == off:
                                cur[1] = off + 512
                            else:
                                cur = [off, off + 512]
                                runs.append(cur)
                        else:
                            cur = None
                return [(a, b) for a, b in runs]

            def handle_chunk(pair, ch):
                p7t = state.get(("p7t", pair))
                if ch == 1:
                    l8 = ft("l8", pair)
                    nc.vector.tensor_mul(l8, p7t, gtE(pair, 1))
                    r8 = ft("r8", pair)
                    nc.gpsimd.tensor_sub(r8, p7t, l8)
                    tap("gt1", pair, gtE(pair, 1))
                    tap("l8", pair, l8)
                    tap("r8", pair, r8)
                    emit_fold(pair, 0, l8)
                    emit_fold(pair, 1, r8)
                elif ch == 2:
                    l8 = state[("l8", pair)]
                    l9a = ft("l9a", pair)
                    nc.vector.tensor_mul(l9a, l8, gtE(pair, 2))
                    r9a = ft("r9a", pair)
                    nc.gpsimd.tensor_sub(r9a, l8, l9a)
                    tap("l9a", pair, l9a)
                    tap("r9a", pair, r9a)
                    emit_fold(pair, 2, l9a)
                elif ch == 3:
                    r8 = state[("r8", pair)]
                    l9b = ft("l9b", pair)
                    nc.vector.tensor_mul(l9b, r8, gtE(pair, 3))
                    r9b = ft("r9b", pair)
                    nc.gpsimd.tensor_sub(r9b, r8, l9b)
                    emit_fold(pair, 3, l9b)
                elif ch == 7:
                    m_d = ft("m_d", pair)
                    nc.vector.tensor_mul(m_d, state[("r9b", pair)],
                                         gtE(pair, 7))
                    emit_fold(pair, 7, m_d)
                elif ch == 6:
                    # tau-form consumers: m = parent * tau
                    parent = state[("r9a", pair)]
                    tt = state[("t", pair, ch)]
                    m = ft("m_c", pair)
                    pruns = poly_runs(pair, ch)
                    if pruns:
                        u = chain.tile([128, 2 * SLAB], f16, tag="u",
                                       name=f"u{pair}_{ch}")
                        r = chain.tile([128, 2 * SLAB], f16, tag="r",
                                       name=f"r{pair}_{ch}")
                        v = chain.tile([128, 2 * SLAB], f16, tag="v",
                                       name=f"v{pair}_{ch}")
                        pt = chain.tile([128, 2 * SLAB], f16, tag="pt",
                                        name=f"pt{pair}_{ch}")
                    for a, b in pruns:
                        nc.vector.tensor_mul(u[:, a:b], tt[:, a:b], tt[:, a:b])
                        nc.vector.tensor_scalar(r[:, a:b], u[:, a:b],
                                                PC, PB, MULT, ADD)
                        nc.gpsimd.tensor_mul(v[:, a:b], r[:, a:b], u[:, a:b])
                        nc.vector.tensor_mul(pt[:, a:b], parent[:, a:b],
                                             tt[:, a:b])
                        # va = v + PA on DVE tsp; m = va * pt on Pool
                        nc.vector.tensor_scalar(r[:, a:b], v[:, a:b],
                                                PA, 0.0, ADD, ADD)
                        nc.gpsimd.tensor_mul(m[:, a:b], r[:, a:b],
                                             pt[:, a:b])
                    for a, b in exact_runs(pair, ch):
                        nc.vector.tensor_mul(m[:, a:b], parent[:, a:b],
                                             tt[:, a:b])
                    tap("t6", pair, tt)
                    tap("m_c", pair, m)
                    emit_fold(pair, 6 if ch == 6 else 7, m)
                elif ch == 4:
                    m_a = ft("m_a", pair)
                    nc.gpsimd.tensor_mul(m_a, state[("l9a", pair)],
                                         gtE(pair, 4))
                    emit_fold(pair, 4, m_a)

            def handle_ch5_piece(pair, si, h):
                # tail chunk piece-wise for a short drain
                off = si * SLAB + h * 512
                if ("m_b", pair) not in state:
                    ft("m_b", pair)
                m_b = state[("m_b", pair)]
                nc.vector.tensor_mul(m_b[:, off: off + 512],
                                     state[("l9b", pair)][:, off: off + 512],
                                     gtE(pair, 5)[:, off: off + 512])
                regions = [(si, h * 4 + j) for j in range(4)]
                emit_fold(pair, 5, m_b, regions=regions)
                if pair == N_PAIRS - 1 and si == 1:
                    fold_q.append((emit_out_half, (pair, si, h)))
                elif h == 1:
                    fold_q.append((emit_out, (pair, si)))

            # ------------- unified step schedule -------------
            # gt0 slabs 0,1 first; then pair-0 deep steps interleaved with
            # the remaining gt0 steps; then pairs 1-3.  Completion handlers
            # are deferred one step so evacs stay ahead in the DVE queue.
            G_STEPS = _gt0_steps()
            order = [("g", 0), ("g", 1)]
            gi = 2
            for k in range(10):
                order.append(("p", 0, k))
                if gi < len(G_STEPS):
                    order.append(("g", gi))
                    gi += 1
            for pair in range(1, N_PAIRS):
                order += [("p", pair, k) for k in range(10)]

            P_WIDTHS = [SUPER] * 9 + [512]
            P_PIECES = []
            p = 0
            for w in P_WIDTHS:
                P_PIECES.append(PIECE_ORDER[p: p + w // 512])
                p += w // 512

            pending = []

            def flush_pending():
                for fn, args in pending:
                    fn(*args)
                pending.clear()

            def emit_g_step(ref):
                off, width, pieces = G_STEPS[ref]
                with tc.high_priority():
                    zs = zpsum.tile([128, SUPER], f32, tag="zs",
                                    name=f"zs0_{off}")
                    for i, (s, h) in enumerate(pieces):
                        col0 = s * SLAB + h * 512
                        nc.tensor.matmul(
                            zs[:, i * 512: (i + 1) * 512],
                            lhsT=w_sb[:, 0, :],
                            rhs=xt2[:, col0: col0 + 512],
                            start=True, stop=True,
                        )
                    nc.scalar.activation(
                        gt0[:, off: off + width], zs[:, 0:width],
                        SIG, bias=0.0, scale=1.0)
                for sl in (2 * ref + 2, 2 * ref + 3):
                    if sl < N_SLABS and ("xld", sl) not in state:
                        state[("xld", sl)] = True
                        nc.sync.dma_start(
                            out=xt2[:, sl * SLAB: (sl + 1) * SLAB],
                            in_=xt2_d[:, sl * SLAB: (sl + 1) * SLAB])
                flush_folds()
                flush_pending()
                for s, h in pieces:
                    if h == 1:
                        pending.append((emit_shallow, (s,)))

            def emit_p_step(pair, k):
                if k == 0:
                    state[("gtd", pair)] = gtpool.tile(
                        [128, 6 * 2 * SLAB], f16, tag="gtd",
                        name=f"gtd{pair}")
                    state[("t", pair, 6)] = tpool.tile(
                        [128, 2 * SLAB], f16, tag="t6",
                        name=f"t{pair}_6")
                pieces = P_PIECES[k]
                with tc.high_priority():
                    zs = zpsum.tile([128, SUPER], f32, tag="zs",
                                    name=f"zs{pair}_{k}")
                    for i, (ch, si, h) in enumerate(pieces):
                        col0 = (2 * pair + si) * SLAB + h * 512
                        nc.tensor.matmul(
                            zs[:, i * 512: (i + 1) * 512],
                            lhsT=w_sb[:, ch, :],
                            rhs=xt2[:, col0: col0 + 512],
                            start=True, stop=True,
                        )

                    # route contiguous same-destination runs
                    def pdest(pc):
                        ch_, si_, h_ = pc
                        if ch_ in IDXE:
                            return ("g", IDXE[ch_] * 2 * SLAB
                                    + si_ * SLAB + h_ * 512)
                        kind = "p" if pc in POLY_PIECES else "e"
                        return (kind + str(ch_), si_ * SLAB + h_ * 512)

                    i = 0
                    while i < len(pieces):
                        kind, doff = pdest(pieces[i])
                        j = i + 1
                        while j < len(pieces):
                            k2, d2 = pdest(pieces[j])
                            if k2 != kind or d2 != doff + (j - i) * 512:
                                break
                            j += 1
                        nw = (j - i) * 512
                        zsl = zs[:, i * 512: i * 512 + nw]
                        if kind == "g":
                            dst = state[("gtd", pair)]
                            nc.scalar.activation(
                                dst[:, doff: doff + nw], zsl,
                                SIG, bias=0.0, scale=1.0)
                        else:
                            dst = state[("t", pair, int(kind[1]))]
                            if kind[0] == "p":
                                nc.vector.tensor_scalar(
                                    dst[:, doff: doff + nw], zsl,
                                    -1.0, 1.0, MAX, MIN)
                            else:
                                nc.scalar.activation(
                                    dst[:, doff: doff + nw], zsl,
                                    TANH, bias=0.0, scale=T_POLY / 2)
                        i = j
                flush_folds()
                flush_pending()
                for ch, si, h in pieces:
                    if ch == 5:
                        item = (handle_ch5_piece, (pair, si, h))
                    elif si == 1 and h == 1:
                        item = (handle_chunk, (pair, ch))
                    else:
                        continue
                    item[0](*item[1])

            for step in order:
                if step[0] == "g":
                    emit_g_step(step[1])
                else:
                    emit_p_step(step[1], step[2])
            flush_pending()
            flush_folds()

    nc.finalize()
    return nc


def _get_program():
    if "nc" not in _CACHE:
        _CACHE["nc"] = _build_program()
    return _CACHE["nc"]


def kernel(x, c, s, leaf_logits, dims, max_depth):
    from concourse.bass_utils import run_bass_kernel_spmd

    assert int(max_depth) == MAX_DEPTH
    x = np.asarray(x, dtype=F32)
    c = np.asarray(c, dtype=F32)
    s = np.asarray(s, dtype=F32)
    leaf_logits = np.asarray(leaf_logits, dtype=F32)
    dims = np.asarray(dims)

    W, M, lo_row = _build_constants(c, s, dims, leaf_logits)
    wt = np.ascontiguousarray(W.transpose(1, 0, 2))            # [128, 8, 128]
    mt = np.ascontiguousarray(M.transpose(1, 0, 2))            # [128, 8, 16]

    in_maps = []
    for core in range(N_CORES):
        xc = x[core * B_CORE: (core + 1) * B_CORE]             # [8192, 64]
        xT = np.ascontiguousarray(xc.T).astype(F32)            # [64, 8192]
        x_hi = xT.astype(F16)
        x_lo = (xT - x_hi.astype(F32)).astype(F16)
        xt2 = np.empty((128, B_CORE), dtype=F16)
        xt2[:IN_DIM] = x_hi
        for d, r in lo_row.items():
            xt2[r] = x_lo[d]
        xt2[126] = 1.0
        xt2[127] = 1.0
        in_maps.append({"xt2": xt2, "wt": wt, "mt": mt})

    _CACHE["in_maps"] = in_maps
    nc = _get_program()
    res = run_bass_kernel_spmd(nc, in_maps, core_ids=list(range(N_CORES)))

    out = np.empty((B_FULL, N_CLASS), dtype=F32)
    for core in range(N_CORES):
        outt = res.results[core]["outt"]                       # [128, 1024]
        for sl in range(N_SLABS):
            blk = outt[:, sl * 128: sl * 128 + 128]            # [128p, 8j*16c]
            blk = blk.reshape(128, 8, 16)[:, :, :N_CLASS]      # [p, j, cl]
            dst = out[core * B_CORE + sl * SLAB:
                      core * B_CORE + (sl + 1) * SLAB]
            dst.reshape(8, 128, N_CLASS)[...] = (
                blk.transpose(1, 0, 2).astype(F32))
    return out


# revision 34
# speedup vs baseline: 1.0405x; 1.0405x over previous
"""Trainium2 Bass kernel for nn_DFPT_Node (soft binary decision tree).

Full inputs in, full output out; internally data-parallel over 8 NeuronCores
(batch sharded 65536 -> 8 x 8192). Tree params (c, s, dims, leaf_logits) are
baked into compiled constants on the host.

  gate:  g = sigmoid(-4 (x[:,dims] - c)/|s|) = sigmoid(a*x + b) via a scaled
         one-hot matmul with K=128 = [x_hi(64) | x_lo(62 dims) | 1 | 1]; the
         last two rows carry b = b_hi + b_lo (f16 split, ~22 bits), so the
         sigmoid needs no per-chunk bias and one ACT instruction can span
         chunk boundaries (1536-wide supertiles, fewer ACT init charges).
         The two dims whose sharpest gate is softest lose their x_lo row
         (slope <= ~20, error ~1e-3 in z; harmless).
  tree:  levels 0-6 batch-major (batch on partitions), levels 7-9 node-major
         (nodes on partitions, batch on free dim) in block (bit-reversed)
         leaf order; level 9 folded into the output matmul with an 8-chunk
         basis F = [l8, r8, l9a, l9b, u0, u1, q2, q4] (q2 = r9a*g, q4 =
         r9b*g via explicit r9a/r9b subtractions - 2 fewer PSUM chunks than
         the 10-chunk basis at the same DVE op count).
  sched: event-driven software pipeline at chunk granularity: each sigmoid
         supertile completion triggers exactly the newly-unblocked shallow /
         deep / fold work, so the drain after the last sigmoid is short.

Output leaves the device as outT [10->16, B_core] packed 4 slabs per 128
partitions; host transposes back.
"""

import numpy as np

B_FULL = 65536
IN_DIM = 64
N_CLASS = 10
MAX_DEPTH = 10
N_CORES = 8
B_CORE = B_FULL // N_CORES      # 8192
SLAB = 1024                     # batch columns per slab
N_SLABS = B_CORE // SLAB        # 8
N_CHUNKS = 8                    # node-major chunks of 128 nodes
N_FCHUNKS = 8                   # fold basis chunks
SUPER = 1536                    # sigmoid supertile width (3 psum banks)
PAIR_FLAT = 2 * N_CHUNKS * SLAB          # 16384 flat gt elems per slab pair
STEPS_PER_PAIR = (PAIR_FLAT + SUPER - 1) // SUPER  # 11
N_PAIRS = N_SLABS // 2

F16 = np.float16
F32 = np.float32

_CACHE = {}


def _build_tree_layout():
    """pos[d][i] = reference position within level d of block-order index i."""
    pos = [np.array([0], dtype=np.int64)]
    for _ in range(MAX_DEPTH):
        p = pos[-1]
        pos.append(np.concatenate([2 * p, 2 * p + 1]))
    return pos


def _build_constants(c, s, dims, leaf_logits):
    """W chunks [8,128,128] f16 (bias folded in rows 126/127), M [8,128,10]."""
    pos = _build_tree_layout()
    chunk_nodes = -np.ones((N_CHUNKS, 128), dtype=np.int64)
    for d in range(7):
        base = (1 << d) - 1
        chunk_nodes[0, base: base + (1 << d)] = base + pos[d]
    chunk_nodes[1, :] = 127 + pos[7]
    lvl8 = 255 + pos[8]
    chunk_nodes[2, :] = lvl8[:128]
    chunk_nodes[3, :] = lvl8[128:]
    lvl9 = 511 + pos[9]
    for t in range(4):
        chunk_nodes[4 + t, :] = lvl9[128 * t: 128 * (t + 1)]

    a64 = -4.0 / np.abs(s.astype(np.float64))
    a16 = a64.astype(F16)
    b64 = -a16.astype(np.float64) * c.astype(np.float64)
    b_hi = b64.astype(F16)
    b_lo = (b64 - b_hi.astype(np.float64)).astype(F16)

    # the two dims whose sharpest gate is softest lose their x_lo row
    min_s = np.full(IN_DIM, np.inf)
    for g in range(len(dims)):
        d = int(dims[g])
        min_s[d] = min(min_s[d], abs(float(s[g])))
    drop = np.argsort(-min_s)[:2]
    lo_row = {}
    r = IN_DIM
    for d in range(IN_DIM):
        if d not in drop:
            lo_row[d] = r
            r += 1
    assert r == 126

    W = np.zeros((N_CHUNKS, 128, 128), dtype=F16)
    ch_idx, col_idx = np.nonzero(chunk_nodes >= 0)
    g_idx = chunk_nodes[ch_idx, col_idx]
    for ch, col, g in zip(ch_idx, col_idx, g_idx):
        d = int(dims[g])
        W[ch, d, col] = a16[g]
        if d in lo_row:
            W[ch, lo_row[d], col] = a16[g]
        W[ch, 126, col] = b_hi[g]
        W[ch, 127, col] = b_lo[g]

    L_my = leaf_logits[pos[MAX_DEPTH]].astype(np.float64)  # [1024, 10] block
    A = L_my[:512] - L_my[512:]
    Bm = L_my[512:]
    At = [A[128 * t: 128 * (t + 1)] for t in range(4)]
    Bt = [Bm[128 * t: 128 * (t + 1)] for t in range(4)]
    # F basis: [l8, r8, l9a, l9b, u0=l9a*g9a, u1=l9b*g9b, q2=r9a*g9c,
    #           q4=r9b*g9d] with r9a = l8-l9a, r9b = r8-l9b:
    # out = l8 B2 + r8 B3 + l9a (B0-B2) + l9b (B1-B3) + u0 A0 + u1 A1
    #       + q2 A2 + q4 A3
    Mlist = [Bt[2], Bt[3], Bt[0] - Bt[2], Bt[1] - Bt[3],
             At[0], At[1], At[2], At[3]]
    M = np.zeros((N_FCHUNKS, 128, N_CLASS), dtype=F16)
    for i, m in enumerate(Mlist):
        M[i] = m.astype(F16)
    return W, M, lo_row


def _step_table():
    """Per sigmoid step: (gt dest, offset, width, z pieces, completions).

    Phase 1 evaluates chunk 0 (shallow gates) of every slab into gt0 so all
    cascades and transposes run early; phase 2 evaluates chunks 1-7
    slab-major into per-pair gtd tiles. A piece is 512 batch columns
    [h*512, h*512+512) of one chunk of one slab.
    """
    steps = []
    p = 0
    off = 0
    for w in (512, 1536, 1536, 1536, 1536, 1536):
        n = w // 512
        pieces = [(q // 2, 0, q % 2) for q in range(p, p + n)]
        done = [(q // 2, 0, 1) for q in range(p, p + n) if q % 2 == 1]
        steps.append(("gt0", None, off, w, pieces, done))
        p += n
        off += w
    for pair in range(N_PAIRS):
        order = [(2 * pair + si, 1 + c, h)
                 for si in range(2) for c in range(7) for h in range(2)]
        p = 0
        off = 0
        for k, w in enumerate([1536] * 9 + [512]):
            n = w // 512
            pieces = order[p: p + n]
            done = [(s, ch, h) for s, ch, h in pieces
                    if h == 1 or ch == 7]
            steps.append(("gtd", pair, off, w, pieces, done))
            p += n
            off += w
    return steps


def _build_program():
    import concourse.bass as bass
    import concourse.tile as tile
    from concourse import bacc, mybir

    f16 = mybir.dt.float16
    f32 = mybir.dt.float32
    SIG = mybir.ActivationFunctionType.Sigmoid

    nc = bacc.Bacc("TRN2", target_bir_lowering=False)
    xt2_d = nc.dram_tensor("xt2", [128, B_CORE], f16, kind="ExternalInput")
    wt_d = nc.dram_tensor("wt", [128, N_CHUNKS, 128], f16, kind="ExternalInput")
    mt_d = nc.dram_tensor("mt", [128, N_FCHUNKS, 16], f16, kind="ExternalInput")
    out_d = nc.dram_tensor("outt", [128, B_CORE // 4], f16, kind="ExternalOutput")

    steps = _step_table()

    with tile.TileContext(nc) as tc:
        with (
            tc.tile_pool(name="singles", bufs=1) as singles,
            tc.tile_pool(name="gtpool", bufs=3) as gtpool,
            tc.tile_pool(name="work", bufs=2) as work,
            tc.tile_pool(name="fwork", bufs=2) as fwork,
            tc.tile_pool(name="zpsum", bufs=2, space="PSUM") as zpsum,
            tc.tile_pool(name="opsum", bufs=1, space="PSUM") as opsum,
        ):
            # resident constants; ordered so step 0's z can start ASAP
            w_sb = singles.tile([128, N_CHUNKS, 128], f16)
            nc.sync.dma_start(out=w_sb[:, 0:1, :], in_=wt_d[:, 0:1, :])
            xt2 = singles.tile([128, B_CORE], f16)
            nc.scalar.dma_start(out=xt2[:, 0:512], in_=xt2_d[:, 0:512])
            nc.sync.dma_start(out=xt2[:, 512:2048], in_=xt2_d[:, 512:2048])
            for sl in range(2, N_SLABS):
                t = bass.ts(sl, SLAB)
                nc.sync.dma_start(out=xt2[:, t], in_=xt2_d[:, t])
            nc.sync.dma_start(out=w_sb[:, 1:, :], in_=wt_d[:, 1:, :])
            ones0 = singles.tile([128, 8, 1], f16)
            nc.vector.memset(ones0, 1.0)
            # PE p-state warmup: keep the tensor engine busy from t~0 so the
            # first real matmuls run above the cold clock
            pewarm = singles.tile([128, 512], f16)
            nc.vector.memset(pewarm, 0.0)
            for _ in range(4):
                zw = zpsum.tile([128, SUPER], f32, tag="zs", name="zwarm")
                nc.tensor.matmul(zw[:, 0:512], lhsT=pewarm[:, 0:128],
                                 rhs=pewarm, start=True, stop=True)
            sigwarm = singles.tile([1, 1], f16)
            nc.vector.memset(sigwarm, 0.0)
            nc.scalar.activation(sigwarm, sigwarm, SIG, bias=0.0, scale=1.0)
            m_sb = singles.tile([128, N_FCHUNKS, 16], f16)
            nc.sync.dma_start(out=m_sb, in_=mt_d[:, :, :])
            gt0 = singles.tile([128, N_SLABS * SLAB], f16)

            state = {}

            def emit_shallow(s):
                # chunk-0 gates batch-major via DMA xbar transpose
                gsh = work.tile([128, 8, 128], f16, tag="gsh", name=f"gsh{s}")
                for g in range(8):
                    nc.sync.dma_start(
                        out=gsh[:, g, :],
                        in_=gt0[:, s * SLAB + g * 128: s * SLAB + (g + 1) * 128],
                        transpose=True,
                    )
                # shallow cascade (batch-major, block layout, groups stacked)
                prev = ones0[:, 0:8, :]
                for d in range(7):
                    n = 1 << d
                    cur = work.tile([128, 8, 2 * n], f16, tag=f"pb{d + 1}",
                                    name=f"pb{d + 1}_{s}")
                    gl = gsh[:, :, n - 1: 2 * n - 1]
                    nc.vector.tensor_mul(cur[:, :, 0:n], prev, gl)
                    nc.vector.tensor_sub(cur[:, :, n: 2 * n], prev,
                                         cur[:, :, 0:n])
                    prev = cur
                # p7 batch-major -> node-major via DMA xbar transpose
                p7t = work.tile([128, SLAB], f16, tag="p7t", name=f"p7t{s}")
                for g in range(8):
                    nc.sync.dma_start(
                        out=p7t[:, g * 128: (g + 1) * 128],
                        in_=prev[:, g, :],
                        transpose=True,
                    )
                state[("p7t", s)] = p7t

            def gtc(s, ch):
                gt = state[("gtd", s // 2)]
                base = ((s % 2) * 7 + ch - 1) * SLAB
                return gt[:, base: base + SLAB]

            def ft(name, s):
                t = fwork.tile([128, SLAB], f16, tag=name, name=f"{name}_{s}")
                state[(name, s)] = t
                return t

            FNAMES = ("l8", "r8", "l9a", "l9b", "u0", "u1", "q2", "q4")

            def emit_fold(s, f, halves=(0, 1)):
                # fold chunk f of slab s into op_h[32j:32j+16, :]; the two
                # column halves live in separate psum tiles so the final
                # copies of each half start right after that half's last fold
                grp, j = divmod(s, 4)
                if f == 0 and j == 0:
                    for h in range(2):
                        state["op", h] = opsum.tile(
                            [128, 512], f32, tag=f"op{h}", name=f"op{grp}_{h}")
                src = state[(FNAMES[f], s)]
                for h in halves:
                    nc.tensor.matmul(
                        state["op", h][32 * j: 32 * j + 16, :],
                        lhsT=m_sb[:, f, :],
                        rhs=src[:, h * 512: h * 512 + 512],
                        start=(f == 0),
                        stop=(f == N_FCHUNKS - 1),
                        tile_position=(0, 32 * j),
                    )

            def emit_group_out(grp):
                last = grp == N_SLABS // 4 - 1
                for h in range(2):
                    osb = work.tile([128, 512], f16, tag="osb",
                                    name=f"osb{grp}_{h}")
                    if last and h == 1:
                        # final half: copy on the now-idle ACT engine and
                        # dispatch its DMA from the ACT queue so it does not
                        # serialize behind h0's SP dispatch
                        nc.scalar.copy(osb, state["op", h])
                        q = nc.scalar
                    else:
                        nc.vector.tensor_copy(osb, state["op", h])
                        q = nc.sync
                    q.dma_start(
                        out=out_d[:, grp * SLAB + h * 512:
                                  grp * SLAB + h * 512 + 512],
                        in_=osb,
                    )

            def handle(s, ch, h):
                if ch == 0:
                    emit_shallow(s)
                elif ch == 1:
                    p7t = state[("p7t", s)]
                    l8 = ft("l8", s)
                    nc.vector.tensor_mul(l8, p7t, gtc(s, 1))
                    r8 = ft("r8", s)
                    nc.vector.tensor_sub(r8, p7t, l8)
                elif ch == 2:
                    l8 = state[("l8", s)]
                    l9a = ft("l9a", s)
                    nc.vector.tensor_mul(l9a, l8, gtc(s, 2))
                    r9a = ft("r9a", s)
                    nc.vector.tensor_sub(r9a, l8, l9a)
                    emit_fold(s, 0)
                    emit_fold(s, 1)
                elif ch == 3:
                    r8 = state[("r8", s)]
                    l9b = ft("l9b", s)
                    nc.vector.tensor_mul(l9b, r8, gtc(s, 3))
                    r9b = ft("r9b", s)
                    nc.vector.tensor_sub(r9b, r8, l9b)
                    emit_fold(s, 2)
                elif ch == 4:
                    u0 = ft("u0", s)
                    nc.gpsimd.tensor_mul(u0, state[("l9a", s)], gtc(s, 4))
                    emit_fold(s, 3)
                elif ch == 5:
                    u1 = ft("u1", s)
                    nc.gpsimd.tensor_mul(u1, state[("l9b", s)], gtc(s, 5))
                    emit_fold(s, 4)
                elif ch == 6:
                    q2 = ft("q2", s)
                    nc.vector.tensor_mul(q2, state[("r9a", s)], gtc(s, 6))
                    emit_fold(s, 5)
                elif ch == 7:
                    if h == 0:
                        flush_fold7()
                        q4 = ft("q4", s)
                        nc.gpsimd.tensor_mul(q4[:, 0:512],
                                             state[("r9b", s)][:, 0:512],
                                             gtc(s, 7)[:, 0:512])
                        emit_fold(s, 6)
                        if s == N_SLABS - 1:
                            emit_fold(s, 7, halves=(0,))
                    else:
                        q4 = state[("q4", s)]
                        nc.vector.tensor_mul(q4[:, 512:1024],
                                             state[("r9b", s)][:, 512:1024],
                                             gtc(s, 7)[:, 512:1024])
                        if s == N_SLABS - 1:
                            # drain: no later step will flush; emit now
                            emit_fold(s, 7, halves=(1,))
                            emit_group_out(s // 4)
                        else:
                            state["fold7"] = s

            def flush_fold7():
                s = state.pop("fold7", None)
                if s is not None:
                    emit_fold(s, 7)
                    if s % 4 == 3:
                        emit_group_out(s // 4)

            for ti, (dest, pair, off, width, pieces, done) in enumerate(steps):
                with tc.high_priority():
                    if dest == "gt0":
                        gt = gt0
                    else:
                        if off == 0:
                            state[("gtd", pair)] = gtpool.tile(
                                [128, 2 * 7 * SLAB], f16, tag="gtd",
                                name=f"gtd{pair}")
                        gt = state[("gtd", pair)]
                    zs = zpsum.tile([128, SUPER], f32, tag="zs",
                                    name=f"zs{ti}")
                    for i, (s, ch, h) in enumerate(pieces):
                        col0 = s * SLAB + h * 512
                        nc.tensor.matmul(
                            zs[:, i * 512: (i + 1) * 512],
                            lhsT=w_sb[:, ch, :],
                            rhs=xt2[:, col0: col0 + 512],
                            start=True,
                            stop=True,
                        )
                    nc.scalar.activation(
                        gt[:, off: off + width],
                        zs[:, 0:width], SIG, bias=0.0, scale=1.0,
                    )
                flush_fold7()
                for s, ch, h in done:
                    handle(s, ch, h)
            flush_fold7()

    nc.finalize()
    return nc


def _get_program():
    if "nc" not in _CACHE:
        _CACHE["nc"] = _build_program()
    return _CACHE["nc"]


def kernel(x, c, s, leaf_logits, dims, max_depth):
    from concourse.bass_utils import run_bass_kernel_spmd

    assert int(max_depth) == MAX_DEPTH
    x = np.asarray(x, dtype=F32)
    c = np.asarray(c, dtype=F32)
    s = np.asarray(s, dtype=F32)
    leaf_logits = np.asarray(leaf_logits, dtype=F32)
    dims = np.asarray(dims)

    W, M, lo_row = _build_constants(c, s, dims, leaf_logits)
    wt = np.ascontiguousarray(W.transpose(1, 0, 2))            # [128, 8, 128]
    mt = np.zeros((128, N_FCHUNKS, 16), dtype=F16)
    mt[:, :, :N_CLASS] = M.transpose(1, 0, 2)

    in_maps = []
    for core in range(N_CORES):
        xc = x[core * B_CORE: (core + 1) * B_CORE]             # [8192, 64]
        xT = np.ascontiguousarray(xc.T).astype(F32)            # [64, 8192]
        x_hi = xT.astype(F16)
        x_lo = (xT - x_hi.astype(F32)).astype(F16)
        xt2 = np.empty((128, B_CORE), dtype=F16)
        xt2[:IN_DIM] = x_hi
        for d, r in lo_row.items():
            xt2[r] = x_lo[d]
        xt2[126] = 1.0
        xt2[127] = 1.0
        in_maps.append({"xt2": xt2, "wt": wt, "mt": mt})

    _CACHE["in_maps"] = in_maps
    nc = _get_program()
    res = run_bass_kernel_spmd(nc, in_maps, core_ids=list(range(N_CORES)))

    out = np.empty((B_FULL, N_CLASS), dtype=F32)
    for core in range(N_CORES):
        outt = res.results[core]["outt"]                       # [128, 2048]
        for sl in range(N_SLABS):
            g, j = divmod(sl, 4)
            blk = outt[32 * j: 32 * j + N_CLASS, g * SLAB: (g + 1) * SLAB]
            out[core * B_CORE + sl * SLAB: core * B_CORE + (sl + 1) * SLAB] = (
                blk.T.astype(F32))
    return out



# revision 35
# speedup vs baseline: 1.0486x; 1.0078x over previous
"""Trainium2 Bass kernel for nn_DFPT_Node (soft binary decision tree).

Full inputs in, full output out; internally data-parallel over 8 NeuronCores
(batch sharded 65536 -> 8 x 8192). Tree params (c, s, dims, leaf_logits) are
baked into compiled constants on the host.

  gate:  g = sigmoid(-4 (x[:,dims] - c)/|s|) = sigmoid(a*x + b) via a scaled
         one-hot matmul with K=128 = [x_hi(64) | x_lo(62 dims) | 1 | 1]; the
         last two rows carry b = b_hi + b_lo (f16 split, ~22 bits), so the
         sigmoid needs no per-chunk bias and one ACT instruction can span
         chunk boundaries (1536-wide supertiles, fewer ACT init charges).
         The two dims whose sharpest gate is softest lose their x_lo row
         (slope <= ~20, error ~1e-3 in z; harmless).
  tree:  levels 0-6 batch-major (batch on partitions), levels 7-9 node-major
         (nodes on partitions, batch on free dim) in block (bit-reversed)
         leaf order; level 9 folded into the output matmul with an 8-chunk
         basis F = [l8, r8, l9a, l9b, u0, u1, q2, q4] (q2 = r9a*g, q4 =
         r9b*g via explicit r9a/r9b subtractions - 2 fewer PSUM chunks than
         the 10-chunk basis at the same DVE op count).
  sched: event-driven software pipeline at chunk granularity: each sigmoid
         supertile completion triggers exactly the newly-unblocked shallow /
         deep / fold work, so the drain after the last sigmoid is short.

Output leaves the device as outT [10->16, B_core] packed 4 slabs per 128
partitions; host transposes back.
"""

import numpy as np

B_FULL = 65536
IN_DIM = 64
N_CLASS = 10
MAX_DEPTH = 10
N_CORES = 8
B_CORE = B_FULL // N_CORES      # 8192
SLAB = 1024                     # batch columns per slab
N_SLABS = B_CORE // SLAB        # 8
N_CHUNKS = 8                    # node-major chunks of 128 nodes
N_FCHUNKS = 8                   # fold basis chunks
SUPER = 1536                    # sigmoid supertile width (3 psum banks)
PAIR_FLAT = 2 * N_CHUNKS * SLAB          # 16384 flat gt elems per slab pair
STEPS_PER_PAIR = (PAIR_FLAT + SUPER - 1) // SUPER  # 11
N_PAIRS = N_SLABS // 2

F16 = np.float16
F32 = np.float32

_CACHE = {}


def _build_tree_layout():
    """pos[d][i] = reference position within level d of block-order index i."""
    pos = [np.array([0], dtype=np.int64)]
    for _ in range(MAX_DEPTH):
        p = pos[-1]
        pos.append(np.concatenate([2 * p, 2 * p + 1]))
    return pos


def _build_constants(c, s, dims, leaf_logits):
    """W chunks [8,128,128] f16 (bias folded in rows 126/127), M [8,128,10]."""
    pos = _build_tree_layout()
    chunk_nodes = -np.ones((N_CHUNKS, 128), dtype=np.int64)
    for d in range(7):
        base = (1 << d) - 1
        chunk_nodes[0, base: base + (1 << d)] = base + pos[d]
    chunk_nodes[1, :] = 127 + pos[7]
    lvl8 = 255 + pos[8]
    chunk_nodes[2, :] = lvl8[:128]
    chunk_nodes[3, :] = lvl8[128:]
    lvl9 = 511 + pos[9]
    for t in range(4):
        chunk_nodes[4 + t, :] = lvl9[128 * t: 128 * (t + 1)]

    a64 = -4.0 / np.abs(s.astype(np.float64))
    a16 = a64.astype(F16)
    b64 = -a16.astype(np.float64) * c.astype(np.float64)
    b_hi = b64.astype(F16)
    b_lo = (b64 - b_hi.astype(np.float64)).astype(F16)

    # the two dims whose sharpest gate is softest lose their x_lo row
    min_s = np.full(IN_DIM, np.inf)
    for g in range(len(dims)):
        d = int(dims[g])
        min_s[d] = min(min_s[d], abs(float(s[g])))
    drop = np.argsort(-min_s)[:2]
    lo_row = {}
    r = IN_DIM
    for d in range(IN_DIM):
        if d not in drop:
            lo_row[d] = r
            r += 1
    assert r == 126

    W = np.zeros((N_CHUNKS, 128, 128), dtype=F16)
    ch_idx, col_idx = np.nonzero(chunk_nodes >= 0)
    g_idx = chunk_nodes[ch_idx, col_idx]
    for ch, col, g in zip(ch_idx, col_idx, g_idx):
        d = int(dims[g])
        W[ch, d, col] = a16[g]
        if d in lo_row:
            W[ch, lo_row[d], col] = a16[g]
        W[ch, 126, col] = b_hi[g]
        W[ch, 127, col] = b_lo[g]

    L_my = leaf_logits[pos[MAX_DEPTH]].astype(np.float64)  # [1024, 10] block
    A = L_my[:512] - L_my[512:]
    Bm = L_my[512:]
    At = [A[128 * t: 128 * (t + 1)] for t in range(4)]
    Bt = [Bm[128 * t: 128 * (t + 1)] for t in range(4)]
    # F basis: [l8, r8, l9a, l9b, u0=l9a*g9a, u1=l9b*g9b, q2=r9a*g9c,
    #           q4=r9b*g9d] with r9a = l8-l9a, r9b = r8-l9b:
    # out = l8 B2 + r8 B3 + l9a (B0-B2) + l9b (B1-B3) + u0 A0 + u1 A1
    #       + q2 A2 + q4 A3
    Mlist = [Bt[2], Bt[3], Bt[0] - Bt[2], Bt[1] - Bt[3],
             At[0], At[1], At[2], At[3]]
    M = np.zeros((N_FCHUNKS, 128, N_CLASS), dtype=F16)
    for i, m in enumerate(Mlist):
        M[i] = m.astype(F16)
    return W, M, lo_row


def _step_table():
    """Per sigmoid step: (gt dest, offset, width, z pieces, completions).

    Phase 1 evaluates chunk 0 (shallow gates) of every slab into gt0 so all
    cascades and transposes run early; phase 2 evaluates chunks 1-7
    slab-major into per-pair gtd tiles. A piece is 512 batch columns
    [h*512, h*512+512) of one chunk of one slab.
    """
    steps = []
    p = 0
    off = 0
    for w in (512, 1536, 1536, 1536, 1536, 1536):
        n = w // 512
        pieces = [(q // 2, 0, q % 2) for q in range(p, p + n)]
        done = [(q // 2, 0, 1) for q in range(p, p + n) if q % 2 == 1]
        steps.append(("gt0", None, off, w, pieces, done))
        p += n
        off += w
    for pair in range(N_PAIRS):
        order = [(2 * pair + si, 1 + c, h)
                 for si in range(2) for c in range(7) for h in range(2)]
        p = 0
        off = 0
        for k, w in enumerate([1536] * 9 + [512]):
            n = w // 512
            pieces = order[p: p + n]
            done = [(s, ch, h) for s, ch, h in pieces
                    if h == 1 or ch == 7]
            steps.append(("gtd", pair, off, w, pieces, done))
            p += n
            off += w
    return steps


def _build_program():
    import concourse.bass as bass
    import concourse.tile as tile
    from concourse import bacc, mybir

    f16 = mybir.dt.float16
    f32 = mybir.dt.float32
    SIG = mybir.ActivationFunctionType.Sigmoid

    nc = bacc.Bacc("TRN2", target_bir_lowering=False)
    xt2_d = nc.dram_tensor("xt2", [128, B_CORE], f16, kind="ExternalInput")
    wt_d = nc.dram_tensor("wt", [128, N_CHUNKS, 128], f16, kind="ExternalInput")
    mt_d = nc.dram_tensor("mt", [128, N_FCHUNKS, 16], f16, kind="ExternalInput")
    out_d = nc.dram_tensor("outt", [128, 1024], f16, kind="ExternalOutput")

    steps = _step_table()

    with tile.TileContext(nc) as tc:
        with (
            tc.tile_pool(name="singles", bufs=1) as singles,
            tc.tile_pool(name="gtpool", bufs=3) as gtpool,
            tc.tile_pool(name="work", bufs=2) as work,
            tc.tile_pool(name="fwork", bufs=2) as fwork,
            tc.tile_pool(name="zpsum", bufs=2, space="PSUM") as zpsum,
            tc.tile_pool(name="opsum", bufs=1, space="PSUM") as opsum,
        ):
            # resident constants; ordered so step 0's z can start ASAP
            w_sb = singles.tile([128, N_CHUNKS, 128], f16)
            nc.sync.dma_start(out=w_sb[:, 0:1, :], in_=wt_d[:, 0:1, :])
            xt2 = singles.tile([128, B_CORE], f16)
            nc.scalar.dma_start(out=xt2[:, 0:512], in_=xt2_d[:, 0:512])
            nc.sync.dma_start(out=xt2[:, 512:2048], in_=xt2_d[:, 512:2048])
            for sl in range(2, N_SLABS):
                t = bass.ts(sl, SLAB)
                nc.sync.dma_start(out=xt2[:, t], in_=xt2_d[:, t])
            nc.sync.dma_start(out=w_sb[:, 1:, :], in_=wt_d[:, 1:, :])
            ones0 = singles.tile([128, 8, 1], f16)
            nc.vector.memset(ones0, 1.0)
            # PE p-state warmup: keep the tensor engine busy from t~0 so the
            # first real matmuls run above the cold clock
            pewarm = singles.tile([128, 512], f16)
            nc.vector.memset(pewarm, 0.0)
            for _ in range(4):
                zw = zpsum.tile([128, SUPER], f32, tag="zs", name="zwarm")
                nc.tensor.matmul(zw[:, 0:512], lhsT=pewarm[:, 0:128],
                                 rhs=pewarm, start=True, stop=True)
            sigwarm = singles.tile([1, 1], f16)
            nc.vector.memset(sigwarm, 0.0)
            nc.scalar.activation(sigwarm, sigwarm, SIG, bias=0.0, scale=1.0)
            m_sb = singles.tile([128, N_FCHUNKS, 16], f16)
            nc.sync.dma_start(out=m_sb, in_=mt_d[:, :, :])
            gt0 = singles.tile([128, N_SLABS * SLAB], f16)

            state = {}

            def emit_shallow(s):
                # chunk-0 gates batch-major via DMA xbar transpose
                gsh = work.tile([128, 8, 128], f16, tag="gsh", name=f"gsh{s}")
                for g in range(8):
                    nc.sync.dma_start(
                        out=gsh[:, g, :],
                        in_=gt0[:, s * SLAB + g * 128: s * SLAB + (g + 1) * 128],
                        transpose=True,
                    )
                # shallow cascade (batch-major, block layout, groups stacked)
                prev = ones0[:, 0:8, :]
                for d in range(7):
                    n = 1 << d
                    cur = work.tile([128, 8, 2 * n], f16, tag=f"pb{d + 1}",
                                    name=f"pb{d + 1}_{s}")
                    gl = gsh[:, :, n - 1: 2 * n - 1]
                    nc.vector.tensor_mul(cur[:, :, 0:n], prev, gl)
                    nc.vector.tensor_sub(cur[:, :, n: 2 * n], prev,
                                         cur[:, :, 0:n])
                    prev = cur
                # p7 batch-major -> node-major via DMA xbar transpose
                p7t = work.tile([128, SLAB], f16, tag="p7t", name=f"p7t{s}")
                for g in range(8):
                    nc.sync.dma_start(
                        out=p7t[:, g * 128: (g + 1) * 128],
                        in_=prev[:, g, :],
                        transpose=True,
                    )
                state[("p7t", s)] = p7t

            def gtc(s, ch):
                gt = state[("gtd", s // 2)]
                base = ((s % 2) * 7 + ch - 1) * SLAB
                return gt[:, base: base + SLAB]

            def ft(name, s):
                t = fwork.tile([128, SLAB], f16, tag=name, name=f"{name}_{s}")
                state[(name, s)] = t
                return t

            FNAMES = ("l8", "r8", "l9a", "l9b", "u0", "u1", "q2", "q4")

            def emit_fold(s, f, halves=(0, 1)):
                # flipped fold: for each 128-batch window jw accumulate
                # op[:, j, jw, :] += src[:, jw*128:+128].T @ M[f] (13ns/mm).
                # PSUM start would zero the whole 2KB zero-region, so the
                # tile is memset once and every matmul pure-accumulates.
                grp, j = divmod(s, 4)
                if f == 0 and j == 0:
                    op = opsum.tile([128, 4, 8, 16], f32, tag="op",
                                    name=f"op{grp}")
                    nc.vector.memset(op, 0.0)
                    state["op"] = op
                op = state["op"]
                src = state[(FNAMES[f], s)]
                for h in halves:
                    for w in range(4):
                        jw = h * 4 + w
                        nc.tensor.matmul(
                            op[:, j, jw, :],
                            lhsT=src[:, jw * 128: (jw + 1) * 128],
                            rhs=m_sb[:, f, :],
                            start=False, stop=False,
                            skip_group_check=True,
                        )

            def emit_group_out(grp):
                last = grp == N_SLABS // 4 - 1
                op = state["op"]
                osb = work.tile([128, 512], f16, tag="osb",
                                name=f"osb{grp}")
                if last:
                    # final group: copy on the now-idle ACT engine and
                    # dispatch its DMA from the ACT queue
                    nc.scalar.copy(osb, op[:, :, :, :])
                    q = nc.scalar
                else:
                    nc.vector.tensor_copy(osb, op[:, :, :, :])
                    q = nc.sync
                q.dma_start(
                    out=out_d[:, grp * 512: grp * 512 + 512],
                    in_=osb,
                )

            def handle(s, ch, h):
                if ch == 0:
                    emit_shallow(s)
                elif ch == 1:
                    p7t = state[("p7t", s)]
                    l8 = ft("l8", s)
                    nc.vector.tensor_mul(l8, p7t, gtc(s, 1))
                    r8 = ft("r8", s)
                    nc.vector.tensor_sub(r8, p7t, l8)
                elif ch == 2:
                    l8 = state[("l8", s)]
                    l9a = ft("l9a", s)
                    nc.vector.tensor_mul(l9a, l8, gtc(s, 2))
                    r9a = ft("r9a", s)
                    nc.vector.tensor_sub(r9a, l8, l9a)
                    emit_fold(s, 0)
                    emit_fold(s, 1)
                elif ch == 3:
                    r8 = state[("r8", s)]
                    l9b = ft("l9b", s)
                    nc.vector.tensor_mul(l9b, r8, gtc(s, 3))
                    r9b = ft("r9b", s)
                    nc.vector.tensor_sub(r9b, r8, l9b)
                    emit_fold(s, 2)
                elif ch == 4:
                    u0 = ft("u0", s)
                    nc.gpsimd.tensor_mul(u0, state[("l9a", s)], gtc(s, 4))
                    emit_fold(s, 3)
                elif ch == 5:
                    u1 = ft("u1", s)
                    nc.gpsimd.tensor_mul(u1, state[("l9b", s)], gtc(s, 5))
                    emit_fold(s, 4)
                elif ch == 6:
                    q2 = ft("q2", s)
                    nc.vector.tensor_mul(q2, state[("r9a", s)], gtc(s, 6))
                    emit_fold(s, 5)
                elif ch == 7:
                    if h == 0:
                        flush_fold7()
                        q4 = ft("q4", s)
                        nc.gpsimd.tensor_mul(q4[:, 0:512],
                                             state[("r9b", s)][:, 0:512],
                                             gtc(s, 7)[:, 0:512])
                        emit_fold(s, 6)
                        if s == N_SLABS - 1:
                            emit_fold(s, 7, halves=(0,))
                    else:
                        q4 = state[("q4", s)]
                        nc.vector.tensor_mul(q4[:, 512:1024],
                                             state[("r9b", s)][:, 512:1024],
                                             gtc(s, 7)[:, 512:1024])
                        if s == N_SLABS - 1:
                            # drain: no later step will flush; emit now
                            emit_fold(s, 7, halves=(1,))
                            emit_group_out(s // 4)
                        else:
                            state["fold7"] = s

            def flush_fold7():
                s = state.pop("fold7", None)
                if s is not None:
                    emit_fold(s, 7)
                    if s % 4 == 3:
                        emit_group_out(s // 4)

            for ti, (dest, pair, off, width, pieces, done) in enumerate(steps):
                with tc.high_priority():
                    if dest == "gt0":
                        gt = gt0
                    else:
                        if off == 0:
                            state[("gtd", pair)] = gtpool.tile(
                                [128, 2 * 7 * SLAB], f16, tag="gtd",
                                name=f"gtd{pair}")
                        gt = state[("gtd", pair)]
                    zs = zpsum.tile([128, SUPER], f32, tag="zs",
                                    name=f"zs{ti}")
                    for i, (s, ch, h) in enumerate(pieces):
                        col0 = s * SLAB + h * 512
                        nc.tensor.matmul(
                            zs[:, i * 512: (i + 1) * 512],
                            lhsT=w_sb[:, ch, :],
                            rhs=xt2[:, col0: col0 + 512],
                            start=True,
                            stop=True,
                        )
                    nc.scalar.activation(
                        gt[:, off: off + width],
                        zs[:, 0:width], SIG, bias=0.0, scale=1.0,
                    )
                flush_fold7()
                for s, ch, h in done:
                    handle(s, ch, h)
            flush_fold7()

    nc.finalize()
    return nc


def _get_program():
    if "nc" not in _CACHE:
        _CACHE["nc"] = _build_program()
    return _CACHE["nc"]


def kernel(x, c, s, leaf_logits, dims, max_depth):
    from concourse.bass_utils import run_bass_kernel_spmd

    assert int(max_depth) == MAX_DEPTH
    x = np.asarray(x, dtype=F32)
    c = np.asarray(c, dtype=F32)
    s = np.asarray(s, dtype=F32)
    leaf_logits = np.asarray(leaf_logits, dtype=F32)
    dims = np.asarray(dims)

    W, M, lo_row = _build_constants(c, s, dims, leaf_logits)
    wt = np.ascontiguousarray(W.transpose(1, 0, 2))            # [128, 8, 128]
    mt = np.zeros((128, N_FCHUNKS, 16), dtype=F16)
    mt[:, :, :N_CLASS] = M.transpose(1, 0, 2)

    in_maps = []
    for core in range(N_CORES):
        xc = x[core * B_CORE: (core + 1) * B_CORE]             # [8192, 64]
        xT = np.ascontiguousarray(xc.T).astype(F32)            # [64, 8192]
        x_hi = xT.astype(F16)
        x_lo = (xT - x_hi.astype(F32)).astype(F16)
        xt2 = np.empty((128, B_CORE), dtype=F16)
        xt2[:IN_DIM] = x_hi
        for d, r in lo_row.items():
            xt2[r] = x_lo[d]
        xt2[126] = 1.0
        xt2[127] = 1.0
        in_maps.append({"xt2": xt2, "wt": wt, "mt": mt})

    _CACHE["in_maps"] = in_maps
    nc = _get_program()
    res = run_bass_kernel_spmd(nc, in_maps, core_ids=list(range(N_CORES)))

    out = np.empty((B_FULL, N_CLASS), dtype=F32)
    for core in range(N_CORES):
        outt = res.results[core]["outt"]                       # [128, 1024]
        for sl in range(N_SLABS):
            g, j = divmod(sl, 4)
            blk = outt[:, g * 512 + j * 128: g * 512 + (j + 1) * 128]
            blk = blk.reshape(128, 8, 16)[:, :, :N_CLASS]      # [p, w, cl]
            dst = out[core * B_CORE + sl * SLAB:
                      core * B_CORE + (sl + 1) * SLAB]
            dst.reshape(8, 128, N_CLASS)[...] = (
                blk.transpose(1, 0, 2).astype(F32))
    return out



# revision 36
# speedup vs baseline: 1.0503x; 1.0017x over previous
"""Trainium2 Bass kernel for nn_DFPT_Node (soft binary decision tree).

Full inputs in, full output out; internally data-parallel over 8 NeuronCores
(batch sharded 65536 -> 8 x 8192). Tree params (c, s, dims, leaf_logits) are
baked into compiled constants on the host.

  gate:  g = sigmoid(-4 (x[:,dims] - c)/|s|) = sigmoid(a*x + b) via a scaled
         one-hot matmul with K=128 = [x_hi(64) | x_lo(62 dims) | 1 | 1]; the
         last two rows carry b = b_hi + b_lo (f16 split, ~22 bits), so the
         sigmoid needs no per-chunk bias and one ACT instruction can span
         chunk boundaries (1536-wide supertiles, fewer ACT init charges).
         The two dims whose sharpest gate is softest lose their x_lo row
         (slope <= ~20, error ~1e-3 in z; harmless).
  tree:  levels 0-6 batch-major (batch on partitions), levels 7-9 node-major
         (nodes on partitions, batch on free dim) in block (bit-reversed)
         leaf order; level 9 folded into the output matmul with an 8-chunk
         basis F = [l8, r8, l9a, l9b, u0, u1, q2, q4] (q2 = r9a*g, q4 =
         r9b*g via explicit r9a/r9b subtractions - 2 fewer PSUM chunks than
         the 10-chunk basis at the same DVE op count).
  sched: event-driven software pipeline at chunk granularity: each sigmoid
         supertile completion triggers exactly the newly-unblocked shallow /
         deep / fold work, so the drain after the last sigmoid is short.

Output leaves the device as outT [10->16, B_core] packed 4 slabs per 128
partitions; host transposes back.
"""

import numpy as np

B_FULL = 65536
IN_DIM = 64
N_CLASS = 10
MAX_DEPTH = 10
N_CORES = 8
B_CORE = B_FULL // N_CORES      # 8192
SLAB = 1024                     # batch columns per slab
N_SLABS = B_CORE // SLAB        # 8
N_CHUNKS = 8                    # node-major chunks of 128 nodes
N_FCHUNKS = 8                   # fold basis chunks
SUPER = 1536                    # sigmoid supertile width (3 psum banks)
PAIR_FLAT = 2 * N_CHUNKS * SLAB          # 16384 flat gt elems per slab pair
STEPS_PER_PAIR = (PAIR_FLAT + SUPER - 1) // SUPER  # 11
N_PAIRS = N_SLABS // 2

F16 = np.float16
F32 = np.float32

_CACHE = {}


def _build_tree_layout():
    """pos[d][i] = reference position within level d of block-order index i."""
    pos = [np.array([0], dtype=np.int64)]
    for _ in range(MAX_DEPTH):
        p = pos[-1]
        pos.append(np.concatenate([2 * p, 2 * p + 1]))
    return pos


def _build_constants(c, s, dims, leaf_logits):
    """W chunks [8,128,128] f16 (bias folded in rows 126/127), M [8,128,10]."""
    pos = _build_tree_layout()
    chunk_nodes = -np.ones((N_CHUNKS, 128), dtype=np.int64)
    for d in range(7):
        base = (1 << d) - 1
        chunk_nodes[0, base: base + (1 << d)] = base + pos[d]
    chunk_nodes[1, :] = 127 + pos[7]
    lvl8 = 255 + pos[8]
    chunk_nodes[2, :] = lvl8[:128]
    chunk_nodes[3, :] = lvl8[128:]
    lvl9 = 511 + pos[9]
    for t in range(4):
        chunk_nodes[4 + t, :] = lvl9[128 * t: 128 * (t + 1)]

    a64 = -4.0 / np.abs(s.astype(np.float64))
    a16 = a64.astype(F16)
    b64 = -a16.astype(np.float64) * c.astype(np.float64)
    b_hi = b64.astype(F16)
    b_lo = (b64 - b_hi.astype(np.float64)).astype(F16)

    # the two dims whose sharpest gate is softest lose their x_lo row
    min_s = np.full(IN_DIM, np.inf)
    for g in range(len(dims)):
        d = int(dims[g])
        min_s[d] = min(min_s[d], abs(float(s[g])))
    drop = np.argsort(-min_s)[:2]
    lo_row = {}
    r = IN_DIM
    for d in range(IN_DIM):
        if d not in drop:
            lo_row[d] = r
            r += 1
    assert r == 126

    W = np.zeros((N_CHUNKS, 128, 128), dtype=F16)
    ch_idx, col_idx = np.nonzero(chunk_nodes >= 0)
    g_idx = chunk_nodes[ch_idx, col_idx]
    for ch, col, g in zip(ch_idx, col_idx, g_idx):
        d = int(dims[g])
        W[ch, d, col] = a16[g]
        if d in lo_row:
            W[ch, lo_row[d], col] = a16[g]
        W[ch, 126, col] = b_hi[g]
        W[ch, 127, col] = b_lo[g]

    L_my = leaf_logits[pos[MAX_DEPTH]].astype(np.float64)  # [1024, 10] block
    A = L_my[:512] - L_my[512:]
    Bm = L_my[512:]
    At = [A[128 * t: 128 * (t + 1)] for t in range(4)]
    Bt = [Bm[128 * t: 128 * (t + 1)] for t in range(4)]
    # F basis: [l8, r8, l9a, l9b, u0=l9a*g9a, u1=l9b*g9b, q2=r9a*g9c,
    #           q4=r9b*g9d] with r9a = l8-l9a, r9b = r8-l9b:
    # out = l8 B2 + r8 B3 + l9a (B0-B2) + l9b (B1-B3) + u0 A0 + u1 A1
    #       + q2 A2 + q4 A3
    Mlist = [Bt[2], Bt[3], Bt[0] - Bt[2], Bt[1] - Bt[3],
             At[0], At[1], At[2], At[3]]
    M = np.zeros((N_FCHUNKS, 128, N_CLASS), dtype=F16)
    for i, m in enumerate(Mlist):
        M[i] = m.astype(F16)
    return W, M, lo_row


def _step_table():
    """Per sigmoid step: (gt dest, offset, width, z pieces, completions).

    Phase 1 evaluates chunk 0 (shallow gates) of every slab into gt0 so all
    cascades and transposes run early; phase 2 evaluates chunks 1-7
    slab-major into per-pair gtd tiles. A piece is 512 batch columns
    [h*512, h*512+512) of one chunk of one slab.
    """
    steps = []
    p = 0
    off = 0
    for w in (512, 2048, 1536, 2048, 1536, 512):
        n = w // 512
        pieces = [(q // 2, 0, q % 2) for q in range(p, p + n)]
        done = [(q // 2, 0, 1) for q in range(p, p + n) if q % 2 == 1]
        steps.append(("gt0", None, off, w, pieces, done))
        p += n
        off += w
    for pair in range(N_PAIRS):
        order = [(2 * pair + si, 1 + c, h)
                 for si in range(2) for c in range(7) for h in range(2)]
        p = 0
        off = 0
        for k, w in enumerate([2048, 1536] * 4):
            n = w // 512
            pieces = order[p: p + n]
            done = [(s, ch, h) for s, ch, h in pieces
                    if h == 1 or ch == 7]
            steps.append(("gtd", pair, off, w, pieces, done))
            p += n
            off += w
    return steps


def _build_program():
    import concourse.bass as bass
    import concourse.tile as tile
    from concourse import bacc, mybir

    f16 = mybir.dt.float16
    f32 = mybir.dt.float32
    SIG = mybir.ActivationFunctionType.Sigmoid

    nc = bacc.Bacc("TRN2", target_bir_lowering=False)
    xt2_d = nc.dram_tensor("xt2", [128, B_CORE], f16, kind="ExternalInput")
    wt_d = nc.dram_tensor("wt", [128, N_CHUNKS, 128], f16, kind="ExternalInput")
    mt_d = nc.dram_tensor("mt", [128, N_FCHUNKS, 16], f16, kind="ExternalInput")
    out_d = nc.dram_tensor("outt", [128, 1024], f16, kind="ExternalOutput")

    steps = _step_table()

    with tile.TileContext(nc) as tc:
        with (
            tc.tile_pool(name="singles", bufs=1) as singles,
            tc.tile_pool(name="gtpool", bufs=3) as gtpool,
            tc.tile_pool(name="work", bufs=2) as work,
            tc.tile_pool(name="fwork", bufs=2) as fwork,
            tc.tile_pool(name="zpsum", bufs=1, space="PSUM") as zpsum,
            tc.tile_pool(name="opsum", bufs=1, space="PSUM") as opsum,
        ):
            # resident constants; ordered so step 0's z can start ASAP
            w_sb = singles.tile([128, N_CHUNKS, 128], f16)
            nc.sync.dma_start(out=w_sb[:, 0:1, :], in_=wt_d[:, 0:1, :])
            xt2 = singles.tile([128, B_CORE], f16)
            nc.scalar.dma_start(out=xt2[:, 0:512], in_=xt2_d[:, 0:512])
            nc.sync.dma_start(out=xt2[:, 512:2048], in_=xt2_d[:, 512:2048])
            for sl in range(2, N_SLABS):
                t = bass.ts(sl, SLAB)
                nc.sync.dma_start(out=xt2[:, t], in_=xt2_d[:, t])
            nc.sync.dma_start(out=w_sb[:, 1:, :], in_=wt_d[:, 1:, :])
            ones0 = singles.tile([128, 8, 1], f16)
            nc.vector.memset(ones0, 1.0)
            # PE p-state warmup: keep the tensor engine busy from t~0 so the
            # first real matmuls run above the cold clock
            pewarm = singles.tile([128, 512], f16)
            nc.vector.memset(pewarm, 0.0)
            for _ in range(4):
                zw = zpsum.tile([128, SUPER], f32, tag="zsB", name="zwarm")
                nc.tensor.matmul(zw[:, 0:512], lhsT=pewarm[:, 0:128],
                                 rhs=pewarm, start=True, stop=True)
            sigwarm = singles.tile([1, 1], f16)
            nc.vector.memset(sigwarm, 0.0)
            nc.scalar.activation(sigwarm, sigwarm, SIG, bias=0.0, scale=1.0)
            m_sb = singles.tile([128, N_FCHUNKS, 16], f16)
            nc.sync.dma_start(out=m_sb, in_=mt_d[:, :, :])
            gt0 = singles.tile([128, N_SLABS * SLAB], f16)

            state = {}

            def emit_shallow(s):
                # chunk-0 gates batch-major via DMA xbar transpose
                gsh = work.tile([128, 8, 128], f16, tag="gsh", name=f"gsh{s}")
                for g in range(8):
                    nc.sync.dma_start(
                        out=gsh[:, g, :],
                        in_=gt0[:, s * SLAB + g * 128: s * SLAB + (g + 1) * 128],
                        transpose=True,
                    )
                # shallow cascade (batch-major, block layout, groups stacked)
                prev = ones0[:, 0:8, :]
                for d in range(7):
                    n = 1 << d
                    cur = work.tile([128, 8, 2 * n], f16, tag=f"pb{d + 1}",
                                    name=f"pb{d + 1}_{s}")
                    gl = gsh[:, :, n - 1: 2 * n - 1]
                    nc.vector.tensor_mul(cur[:, :, 0:n], prev, gl)
                    nc.vector.tensor_sub(cur[:, :, n: 2 * n], prev,
                                         cur[:, :, 0:n])
                    prev = cur
                # p7 batch-major -> node-major via DMA xbar transpose
                p7t = work.tile([128, SLAB], f16, tag="p7t", name=f"p7t{s}")
                for g in range(8):
                    nc.sync.dma_start(
                        out=p7t[:, g * 128: (g + 1) * 128],
                        in_=prev[:, g, :],
                        transpose=True,
                    )
                state[("p7t", s)] = p7t

            def gtc(s, ch):
                gt = state[("gtd", s // 2)]
                base = ((s % 2) * 7 + ch - 1) * SLAB
                return gt[:, base: base + SLAB]

            def ft(name, s):
                t = fwork.tile([128, SLAB], f16, tag=name, name=f"{name}_{s}")
                state[(name, s)] = t
                return t

            FNAMES = ("l8", "r8", "l9a", "l9b", "u0", "u1", "q2", "q4")

            def emit_fold(s, f, halves=(0, 1)):
                # flipped fold: for each 128-batch window jw accumulate
                # op[:, j, jw, :] += src[:, jw*128:+128].T @ M[f] (13ns/mm).
                # PSUM start would zero the whole 2KB zero-region, so the
                # tile is memset once and every matmul pure-accumulates.
                grp, j = divmod(s, 4)
                if f == 0 and j == 0:
                    op = opsum.tile([128, 4, 8, 16], f32, tag="op",
                                    name=f"op{grp}")
                    nc.vector.memset(op, 0.0)
                    state["op"] = op
                op = state["op"]
                src = state[(FNAMES[f], s)]
                for h in halves:
                    for w in range(4):
                        jw = h * 4 + w
                        nc.tensor.matmul(
                            op[:, j, jw, :],
                            lhsT=src[:, jw * 128: (jw + 1) * 128],
                            rhs=m_sb[:, f, :],
                            start=False, stop=False,
                            skip_group_check=True,
                        )

            def emit_group_out(grp):
                last = grp == N_SLABS // 4 - 1
                op = state["op"]
                osb = work.tile([128, 512], f16, tag="osb",
                                name=f"osb{grp}")
                if last:
                    # final group: copy on the now-idle ACT engine and
                    # dispatch its DMA from the ACT queue
                    nc.scalar.copy(osb, op[:, :, :, :])
                    q = nc.scalar
                else:
                    nc.vector.tensor_copy(osb, op[:, :, :, :])
                    q = nc.sync
                q.dma_start(
                    out=out_d[:, grp * 512: grp * 512 + 512],
                    in_=osb,
                )

            def handle(s, ch, h):
                if ch == 0:
                    emit_shallow(s)
                elif ch == 1:
                    p7t = state[("p7t", s)]
                    l8 = ft("l8", s)
                    nc.vector.tensor_mul(l8, p7t, gtc(s, 1))
                    r8 = ft("r8", s)
                    nc.vector.tensor_sub(r8, p7t, l8)
                elif ch == 2:
                    l8 = state[("l8", s)]
                    l9a = ft("l9a", s)
                    nc.vector.tensor_mul(l9a, l8, gtc(s, 2))
                    r9a = ft("r9a", s)
                    nc.vector.tensor_sub(r9a, l8, l9a)
                    emit_fold(s, 0)
                    emit_fold(s, 1)
                elif ch == 3:
                    r8 = state[("r8", s)]
                    l9b = ft("l9b", s)
                    nc.vector.tensor_mul(l9b, r8, gtc(s, 3))
                    r9b = ft("r9b", s)
                    nc.vector.tensor_sub(r9b, r8, l9b)
                    emit_fold(s, 2)
                elif ch == 4:
                    u0 = ft("u0", s)
                    nc.gpsimd.tensor_mul(u0, state[("l9a", s)], gtc(s, 4))
                    emit_fold(s, 3)
                elif ch == 5:
                    u1 = ft("u1", s)
                    nc.gpsimd.tensor_mul(u1, state[("l9b", s)], gtc(s, 5))
                    emit_fold(s, 4)
                elif ch == 6:
                    q2 = ft("q2", s)
                    nc.vector.tensor_mul(q2, state[("r9a", s)], gtc(s, 6))
                    emit_fold(s, 5)
                elif ch == 7:
                    if h == 0:
                        flush_fold7()
                        q4 = ft("q4", s)
                        nc.gpsimd.tensor_mul(q4[:, 0:512],
                                             state[("r9b", s)][:, 0:512],
                                             gtc(s, 7)[:, 0:512])
                        emit_fold(s, 6)
                        if s == N_SLABS - 1:
                            emit_fold(s, 7, halves=(0,))
                    else:
                        q4 = state[("q4", s)]
                        nc.vector.tensor_mul(q4[:, 512:1024],
                                             state[("r9b", s)][:, 512:1024],
                                             gtc(s, 7)[:, 512:1024])
                        if s == N_SLABS - 1:
                            # drain: no later step will flush; emit now
                            emit_fold(s, 7, halves=(1,))
                            emit_group_out(s // 4)
                        else:
                            state["fold7"] = s

            def flush_fold7():
                s = state.pop("fold7", None)
                if s is not None:
                    emit_fold(s, 7)
                    if s % 4 == 3:
                        emit_group_out(s // 4)

            for ti, (dest, pair, off, width, pieces, done) in enumerate(steps):
                with tc.high_priority():
                    if dest == "gt0":
                        gt = gt0
                    else:
                        if off == 0:
                            state[("gtd", pair)] = gtpool.tile(
                                [128, 2 * 7 * SLAB], f16, tag="gtd",
                                name=f"gtd{pair}")
                        gt = state[("gtd", pair)]
                    if width > SUPER:
                        zs = zpsum.tile([128, 2048], f32, tag="zsA",
                                        name=f"zs{ti}")
                    else:
                        zs = zpsum.tile([128, SUPER], f32, tag="zsB",
                                        name=f"zs{ti}")
                    for i, (s, ch, h) in enumerate(pieces):
                        col0 = s * SLAB + h * 512
                        nc.tensor.matmul(
                            zs[:, i * 512: (i + 1) * 512],
                            lhsT=w_sb[:, ch, :],
                            rhs=xt2[:, col0: col0 + 512],
                            start=True,
                            stop=True,
                        )
                    nc.scalar.activation(
                        gt[:, off: off + width],
                        zs[:, 0:width], SIG, bias=0.0, scale=1.0,
                    )
                flush_fold7()
                for s, ch, h in done:
                    handle(s, ch, h)
            flush_fold7()

    nc.finalize()
    return nc


def _get_program():
    if "nc" not in _CACHE:
        _CACHE["nc"] = _build_program()
    return _CACHE["nc"]


def kernel(x, c, s, leaf_logits, dims, max_depth):
    from concourse.bass_utils import run_bass_kernel_spmd

    assert int(max_depth) == MAX_DEPTH
    x = np.asarray(x, dtype=F32)
    c = np.asarray(c, dtype=F32)
    s = np.asarray(s, dtype=F32)
    leaf_logits = np.asarray(leaf_logits, dtype=F32)
    dims = np.asarray(dims)

    W, M, lo_row = _build_constants(c, s, dims, leaf_logits)
    wt = np.ascontiguousarray(W.transpose(1, 0, 2))            # [128, 8, 128]
    mt = np.zeros((128, N_FCHUNKS, 16), dtype=F16)
    mt[:, :, :N_CLASS] = M.transpose(1, 0, 2)

    in_maps = []
    for core in range(N_CORES):
        xc = x[core * B_CORE: (core + 1) * B_CORE]             # [8192, 64]
        xT = np.ascontiguousarray(xc.T).astype(F32)            # [64, 8192]
        x_hi = xT.astype(F16)
        x_lo = (xT - x_hi.astype(F32)).astype(F16)
        xt2 = np.empty((128, B_CORE), dtype=F16)
        xt2[:IN_DIM] = x_hi
        for d, r in lo_row.items():
            xt2[r] = x_lo[d]
        xt2[126] = 1.0
        xt2[127] = 1.0
        in_maps.append({"xt2": xt2, "wt": wt, "mt": mt})

    _CACHE["in_maps"] = in_maps
    nc = _get_program()
    res = run_bass_kernel_spmd(nc, in_maps, core_ids=list(range(N_CORES)))

    out = np.empty((B_FULL, N_CLASS), dtype=F32)
    for core in range(N_CORES):
        outt = res.results[core]["outt"]                       # [128, 1024]
        for sl in range(N_SLABS):
            g, j = divmod(sl, 4)
            blk = outt[:, g * 512 + j * 128: g * 512 + (j + 1) * 128]
            blk = blk.reshape(128, 8, 16)[:, :, :N_CLASS]      # [p, w, cl]
            dst = out[core * B_CORE + sl * SLAB:
                      core * B_CORE + (sl + 1) * SLAB]
            dst.reshape(8, 128, N_CLASS)[...] = (
                blk.transpose(1, 0, 2).astype(F32))
    return out



# revision 38
# speedup vs baseline: 1.0663x; 1.0152x over previous
"""Trainium2 Bass kernel for nn_DFPT_Node (soft binary decision tree).

Full inputs in, full output out; internally data-parallel over 8 NeuronCores
(batch sharded 65536 -> 8 x 8192). Tree params (c, s, dims, leaf_logits) are
baked into compiled constants on the host.

  gate:  g = sigmoid(-4 (x[:,dims] - c)/|s|) = sigmoid(a*x + b) via a scaled
         one-hot matmul with K=128 = [x_hi(64) | x_lo(62 dims) | 1 | 1]; the
         last two rows carry b = b_hi + b_lo (f16 split, ~22 bits), so the
         sigmoid needs no per-chunk bias and one ACT instruction can span
         chunk boundaries (1536-wide supertiles, fewer ACT init charges).
         The two dims whose sharpest gate is softest lose their x_lo row
         (slope <= ~20, error ~1e-3 in z; harmless).
  tree:  levels 0-6 batch-major (batch on partitions), levels 7-9 node-major
         (nodes on partitions, batch on free dim) in block (bit-reversed)
         leaf order; level 9 folded into the output matmul with an 8-chunk
         basis F = [l8, r8, l9a, l9b, u0, u1, q2, q4] (q2 = r9a*g, q4 =
         r9b*g via explicit r9a/r9b subtractions - 2 fewer PSUM chunks than
         the 10-chunk basis at the same DVE op count).
  sched: event-driven software pipeline at chunk granularity: each sigmoid
         supertile completion triggers exactly the newly-unblocked shallow /
         deep / fold work, so the drain after the last sigmoid is short.

Output leaves the device as outT [10->16, B_core] packed 4 slabs per 128
partitions; host transposes back.
"""

import numpy as np

B_FULL = 65536
IN_DIM = 64
N_CLASS = 10
MAX_DEPTH = 10
N_CORES = 8
B_CORE = B_FULL // N_CORES      # 8192
SLAB = 1024                     # batch columns per slab
N_SLABS = B_CORE // SLAB        # 8
N_CHUNKS = 8                    # node-major chunks of 128 nodes
N_FCHUNKS = 8                   # fold basis chunks
SUPER = 1536                    # sigmoid supertile width (3 psum banks)
PAIR_FLAT = 2 * N_CHUNKS * SLAB          # 16384 flat gt elems per slab pair
STEPS_PER_PAIR = (PAIR_FLAT + SUPER - 1) // SUPER  # 11
N_PAIRS = N_SLABS // 2

F16 = np.float16
F32 = np.float32

_CACHE = {}


def _build_tree_layout():
    """pos[d][i] = reference position within level d of block-order index i."""
    pos = [np.array([0], dtype=np.int64)]
    for _ in range(MAX_DEPTH):
        p = pos[-1]
        pos.append(np.concatenate([2 * p, 2 * p + 1]))
    return pos


def _build_constants(c, s, dims, leaf_logits):
    """W chunks [8,128,128] f16 (bias folded in rows 126/127), M [8,128,10]."""
    pos = _build_tree_layout()
    chunk_nodes = -np.ones((N_CHUNKS, 128), dtype=np.int64)
    for d in range(7):
        base = (1 << d) - 1
        chunk_nodes[0, base: base + (1 << d)] = base + pos[d]
    chunk_nodes[1, :] = 127 + pos[7]
    lvl8 = 255 + pos[8]
    chunk_nodes[2, :] = lvl8[:128]
    chunk_nodes[3, :] = lvl8[128:]
    lvl9 = 511 + pos[9]
    for t in range(4):
        chunk_nodes[4 + t, :] = lvl9[128 * t: 128 * (t + 1)]

    a64 = -4.0 / np.abs(s.astype(np.float64))
    a16 = a64.astype(F16)
    b64 = -a16.astype(np.float64) * c.astype(np.float64)
    b_hi = b64.astype(F16)
    b_lo = (b64 - b_hi.astype(np.float64)).astype(F16)

    # the two dims whose sharpest gate is softest lose their x_lo row
    min_s = np.full(IN_DIM, np.inf)
    for g in range(len(dims)):
        d = int(dims[g])
        min_s[d] = min(min_s[d], abs(float(s[g])))
    drop = np.argsort(-min_s)[:2]
    lo_row = {}
    r = IN_DIM
    for d in range(IN_DIM):
        if d not in drop:
            lo_row[d] = r
            r += 1
    assert r == 126

    W = np.zeros((N_CHUNKS, 128, 128), dtype=F16)
    ch_idx, col_idx = np.nonzero(chunk_nodes >= 0)
    g_idx = chunk_nodes[ch_idx, col_idx]
    for ch, col, g in zip(ch_idx, col_idx, g_idx):
        d = int(dims[g])
        W[ch, d, col] = a16[g]
        if d in lo_row:
            W[ch, lo_row[d], col] = a16[g]
        W[ch, 126, col] = b_hi[g]
        W[ch, 127, col] = b_lo[g]

    L_my = leaf_logits[pos[MAX_DEPTH]].astype(np.float64)  # [1024, 10] block
    A = L_my[:512] - L_my[512:]
    Bm = L_my[512:]
    At = [A[128 * t: 128 * (t + 1)] for t in range(4)]
    Bt = [Bm[128 * t: 128 * (t + 1)] for t in range(4)]
    # F basis: [l8, r8, l9a, l9b, u0=l9a*g9a, u1=l9b*g9b, q2=r9a*g9c,
    #           q4=r9b*g9d] with r9a = l8-l9a, r9b = r8-l9b:
    # out = l8 B2 + r8 B3 + l9a (B0-B2) + l9b (B1-B3) + u0 A0 + u1 A1
    #       + q2 A2 + q4 A3
    Mlist = [Bt[2], Bt[3], Bt[0] - Bt[2], Bt[1] - Bt[3],
             At[0], At[1], At[2], At[3]]
    M = np.zeros((N_FCHUNKS, 128, N_CLASS), dtype=F16)
    for i, m in enumerate(Mlist):
        M[i] = m.astype(F16)
    return W, M, lo_row


def _step_table():
    """Per sigmoid step: (gt dest, offset, width, z pieces, completions).

    Phase 1 evaluates chunk 0 (shallow gates) of every slab into gt0 so all
    cascades and transposes run early; phase 2 evaluates chunks 1-7
    slab-major into per-pair gtd tiles. A piece is 512 batch columns
    [h*512, h*512+512) of one chunk of one slab.
    """
    steps = []
    p = 0
    off = 0
    for w, tag in ((512, "B"), (1024, "A"), (1536, "B"), (2048, "A"),
                   (1536, "B"), (1536, "A")):
        n = w // 512
        pieces = [(q // 2, 0, q % 2) for q in range(p, p + n)]
        done = [(q // 2, 0, 1) for q in range(p, p + n) if q % 2 == 1]
        steps.append(("gt0", None, off, w, tag, pieces, done))
        p += n
        off += w
    for pair in range(N_PAIRS):
        order = [(2 * pair + si, 1 + c, h)
                 for si in range(2) for c in range(7) for h in range(2)]
        p = 0
        off = 0
        for k, (w, tag) in enumerate([(1536, "B"), (2048, "A")] * 4):
            n = w // 512
            pieces = order[p: p + n]
            done = [(s, ch, h) for s, ch, h in pieces
                    if h == 1 or ch == 7]
            steps.append(("gtd", pair, off, w, tag, pieces, done))
            p += n
            off += w
    return steps


def _build_program():
    import concourse.bass as bass
    import concourse.tile as tile
    from concourse import bacc, mybir

    f16 = mybir.dt.float16
    f32 = mybir.dt.float32
    SIG = mybir.ActivationFunctionType.Sigmoid

    nc = bacc.Bacc("TRN2", target_bir_lowering=False)
    xt2_d = nc.dram_tensor("xt2", [128, B_CORE], f16, kind="ExternalInput")
    wt_d = nc.dram_tensor("wt", [128, N_CHUNKS, 128], f16, kind="ExternalInput")
    mt_d = nc.dram_tensor("mt", [128, N_FCHUNKS, 16], f16, kind="ExternalInput")
    out_d = nc.dram_tensor("outt", [128, 2, 4, 2, 64], f16,
                           kind="ExternalOutput")

    steps = _step_table()

    with tile.TileContext(nc) as tc:
        with (
            tc.tile_pool(name="singles", bufs=1) as singles,
            tc.tile_pool(name="gtpool", bufs=3) as gtpool,
            tc.tile_pool(name="work", bufs=2) as work,
            tc.tile_pool(name="fwork", bufs=2) as fwork,
            tc.tile_pool(name="zpsum", bufs=1, space="PSUM") as zpsum,
            tc.tile_pool(name="opsum", bufs=1, space="PSUM") as opsum,
        ):
            # resident constants; ordered so step 0's z can start ASAP
            w_sb = singles.tile([128, N_CHUNKS, 128], f16)
            nc.sync.dma_start(out=w_sb[:, 0:1, :], in_=wt_d[:, 0:1, :])
            xt2 = singles.tile([128, B_CORE], f16)
            nc.scalar.dma_start(out=xt2[:, 0:512], in_=xt2_d[:, 0:512])
            nc.sync.dma_start(out=xt2[:, 512:2048], in_=xt2_d[:, 512:2048])
            for sl in range(2, N_SLABS):
                t = bass.ts(sl, SLAB)
                nc.sync.dma_start(out=xt2[:, t], in_=xt2_d[:, t])
            nc.sync.dma_start(out=w_sb[:, 1:, :], in_=wt_d[:, 1:, :])
            ones0 = singles.tile([128, 8, 1], f16)
            nc.vector.memset(ones0, 1.0)
            # PE p-state warmup: keep the tensor engine busy from t~0 so the
            # first real matmuls run above the cold clock
            pewarm = singles.tile([128, 512], f16)
            nc.vector.memset(pewarm, 0.0)
            for _ in range(4):
                zw = zpsum.tile([128, SUPER], f32, tag="zsB", name="zwarm")
                nc.tensor.matmul(zw[:, 0:512], lhsT=pewarm[:, 0:128],
                                 rhs=pewarm, start=True, stop=True)
            sigwarm = singles.tile([1, 1], f16)
            nc.vector.memset(sigwarm, 0.0)
            nc.scalar.activation(sigwarm, sigwarm, SIG, bias=0.0, scale=1.0)
            m_sb = singles.tile([128, N_FCHUNKS, 16], f16)
            nc.sync.dma_start(out=m_sb, in_=mt_d[:, :, :])
            gt0 = singles.tile([128, N_SLABS * SLAB], f16)

            state = {}

            def emit_shallow(s):
                # chunk-0 gates batch-major via DMA xbar transpose
                gsh = work.tile([128, 8, 128], f16, tag="gsh", name=f"gsh{s}")
                for g in range(8):
                    nc.sync.dma_start(
                        out=gsh[:, g, :],
                        in_=gt0[:, s * SLAB + g * 128: s * SLAB + (g + 1) * 128],
                        transpose=True,
                    )
                # shallow cascade (batch-major, block layout, groups stacked)
                prev = ones0[:, 0:8, :]
                for d in range(7):
                    n = 1 << d
                    cur = work.tile([128, 8, 2 * n], f16, tag=f"pb{d + 1}",
                                    name=f"pb{d + 1}_{s}")
                    gl = gsh[:, :, n - 1: 2 * n - 1]
                    nc.vector.tensor_mul(cur[:, :, 0:n], prev, gl)
                    nc.vector.tensor_sub(cur[:, :, n: 2 * n], prev,
                                         cur[:, :, 0:n])
                    prev = cur
                # p7 batch-major -> node-major via DMA xbar transpose
                p7t = work.tile([128, SLAB], f16, tag="p7t", name=f"p7t{s}")
                for g in range(8):
                    nc.sync.dma_start(
                        out=p7t[:, g * 128: (g + 1) * 128],
                        in_=prev[:, g, :],
                        transpose=True,
                    )
                state[("p7t", s)] = p7t

            def gtc(s, ch):
                gt = state[("gtd", s // 2)]
                base = ((s % 2) * 7 + ch - 1) * SLAB
                return gt[:, base: base + SLAB]

            def ft(name, s):
                t = fwork.tile([128, SLAB], f16, tag=name, name=f"{name}_{s}")
                state[(name, s)] = t
                return t

            FNAMES = ("l8", "r8", "l9a", "l9b", "u0", "u1", "q2", "q4")

            def emit_fold(s, f, halves=(0, 1)):
                # flipped fold: for each 128-batch window jw accumulate
                # op[:, j, jw, :] += src[:, jw*128:+128].T @ M[f] (13ns/mm).
                # PSUM start would zero the whole 2KB zero-region, so the
                # tile is memset once and every matmul pure-accumulates.
                grp, j = divmod(s, 4)
                if f == 0 and j == 0:
                    op = opsum.tile([128, 4, 8, 16], f32, tag="op",
                                    name=f"op{grp}")
                    nc.vector.memset(op, 0.0)
                    state["op"] = op
                op = state["op"]
                src = state[(FNAMES[f], s)]
                for h in halves:
                    for w in range(4):
                        jw = h * 4 + w
                        nc.tensor.matmul(
                            op[:, j, jw, :],
                            lhsT=src[:, jw * 128: (jw + 1) * 128],
                            rhs=m_sb[:, f, :],
                            start=False, stop=False,
                            skip_group_check=True,
                        )

            def emit_group_out(grp, halves=(0, 1)):
                last = grp == N_SLABS // 4 - 1
                op = state["op"]
                for h in halves:
                    osb = work.tile([128, 4, 64], f16, tag=f"osb{h}",
                                    name=f"osb{grp}_{h}")
                    if last:
                        # copy on the now-idle ACT engine; dispatch the DMA
                        # from the ACT queue
                        nc.scalar.copy(osb, op[:, :, h * 4: h * 4 + 4, :])
                        q = nc.scalar
                    else:
                        nc.vector.tensor_copy(osb, op[:, :, h * 4: h * 4 + 4, :])
                        q = nc.sync
                    q.dma_start(
                        out=out_d[:, grp, :, h, :],
                        in_=osb,
                    )

            def handle(s, ch, h):
                if ch == 0:
                    emit_shallow(s)
                elif ch == 1:
                    p7t = state[("p7t", s)]
                    l8 = ft("l8", s)
                    nc.vector.tensor_mul(l8, p7t, gtc(s, 1))
                    r8 = ft("r8", s)
                    nc.vector.tensor_sub(r8, p7t, l8)
                elif ch == 2:
                    l8 = state[("l8", s)]
                    l9a = ft("l9a", s)
                    nc.vector.tensor_mul(l9a, l8, gtc(s, 2))
                    r9a = ft("r9a", s)
                    nc.vector.tensor_sub(r9a, l8, l9a)
                    emit_fold(s, 0)
                    emit_fold(s, 1)
                elif ch == 3:
                    r8 = state[("r8", s)]
                    l9b = ft("l9b", s)
                    nc.vector.tensor_mul(l9b, r8, gtc(s, 3))
                    r9b = ft("r9b", s)
                    nc.vector.tensor_sub(r9b, r8, l9b)
                    emit_fold(s, 2)
                elif ch == 4:
                    u0 = ft("u0", s)
                    nc.gpsimd.tensor_mul(u0, state[("l9a", s)], gtc(s, 4))
                    emit_fold(s, 3)
                elif ch == 5:
                    u1 = ft("u1", s)
                    nc.gpsimd.tensor_mul(u1, state[("l9b", s)], gtc(s, 5))
                    emit_fold(s, 4)
                elif ch == 6:
                    q2 = ft("q2", s)
                    nc.vector.tensor_mul(q2, state[("r9a", s)], gtc(s, 6))
                    emit_fold(s, 5)
                elif ch == 7:
                    if h == 0:
                        flush_fold7()
                        q4 = ft("q4", s)
                        nc.gpsimd.tensor_mul(q4[:, 0:512],
                                             state[("r9b", s)][:, 0:512],
                                             gtc(s, 7)[:, 0:512])
                        emit_fold(s, 6)
                        if s == N_SLABS - 1:
                            emit_fold(s, 7, halves=(0,))
                            emit_group_out(s // 4, halves=(0,))
                    else:
                        q4 = state[("q4", s)]
                        nc.vector.tensor_mul(q4[:, 512:1024],
                                             state[("r9b", s)][:, 512:1024],
                                             gtc(s, 7)[:, 512:1024])
                        if s == N_SLABS - 1:
                            # drain: no later step will flush; emit now
                            emit_fold(s, 7, halves=(1,))
                            emit_group_out(s // 4, halves=(1,))
                        else:
                            state["fold7"] = s

            def flush_fold7():
                s = state.pop("fold7", None)
                if s is not None:
                    emit_fold(s, 7)
                    if s % 4 == 3:
                        emit_group_out(s // 4)

            for ti, (dest, pair, off, width, wtag, pieces, done) in enumerate(steps):
                with tc.high_priority():
                    if dest == "gt0":
                        gt = gt0
                    else:
                        if off == 0:
                            state[("gtd", pair)] = gtpool.tile(
                                [128, 2 * 7 * SLAB], f16, tag="gtd",
                                name=f"gtd{pair}")
                        gt = state[("gtd", pair)]
                    if wtag == "A":
                        zs = zpsum.tile([128, 2048], f32, tag="zsA",
                                        name=f"zs{ti}")
                    else:
                        zs = zpsum.tile([128, SUPER], f32, tag="zsB",
                                        name=f"zs{ti}")
                    for i, (s, ch, h) in enumerate(pieces):
                        col0 = s * SLAB + h * 512
                        nc.tensor.matmul(
                            zs[:, i * 512: (i + 1) * 512],
                            lhsT=w_sb[:, ch, :],
                            rhs=xt2[:, col0: col0 + 512],
                            start=True,
                            stop=True,
                        )
                    nc.scalar.activation(
                        gt[:, off: off + width],
                        zs[:, 0:width], SIG, bias=0.0, scale=1.0,
                    )
                flush_fold7()
                for s, ch, h in done:
                    handle(s, ch, h)
            flush_fold7()

    nc.finalize()
    return nc


def _get_program():
    if "nc" not in _CACHE:
        _CACHE["nc"] = _build_program()
    return _CACHE["nc"]


def kernel(x, c, s, leaf_logits, dims, max_depth):
    from concourse.bass_utils import run_bass_kernel_spmd

    assert int(max_depth) == MAX_DEPTH
    x = np.asarray(x, dtype=F32)
    c = np.asarray(c, dtype=F32)
    s = np.asarray(s, dtype=F32)
    leaf_logits = np.asarray(leaf_logits, dtype=F32)
    dims = np.asarray(dims)

    W, M, lo_row = _build_constants(c, s, dims, leaf_logits)
    wt = np.ascontiguousarray(W.transpose(1, 0, 2))            # [128, 8, 128]
    mt = np.zeros((128, N_FCHUNKS, 16), dtype=F16)
    mt[:, :, :N_CLASS] = M.transpose(1, 0, 2)

    in_maps = []
    for core in range(N_CORES):
        xc = x[core * B_CORE: (core + 1) * B_CORE]             # [8192, 64]
        xT = np.ascontiguousarray(xc.T).astype(F32)            # [64, 8192]
        x_hi = xT.astype(F16)
        x_lo = (xT - x_hi.astype(F32)).astype(F16)
        xt2 = np.empty((128, B_CORE), dtype=F16)
        xt2[:IN_DIM] = x_hi
        for d, r in lo_row.items():
            xt2[r] = x_lo[d]
        xt2[126] = 1.0
        xt2[127] = 1.0
        in_maps.append({"xt2": xt2, "wt": wt, "mt": mt})

    _CACHE["in_maps"] = in_maps
    nc = _get_program()
    res = run_bass_kernel_spmd(nc, in_maps, core_ids=list(range(N_CORES)))

    out = np.empty((B_FULL, N_CLASS), dtype=F32)
    for core in range(N_CORES):
        outt = res.results[core]["outt"].reshape(128, 1024)
        for sl in range(N_SLABS):
            g, j = divmod(sl, 4)
            blk = outt[:, g * 512 + j * 128: g * 512 + (j + 1) * 128]
            blk = blk.reshape(128, 8, 16)[:, :, :N_CLASS]      # [p, w, cl]
            dst = out[core * B_CORE + sl * SLAB:
                      core * B_CORE + (sl + 1) * SLAB]
            dst.reshape(8, 128, N_CLASS)[...] = (
                blk.transpose(1, 0, 2).astype(F32))
    return out

